# revision 1
# baseline (speedup 1.0000x reference)
"""Ensemble-SRN MoE routing kernel for 8 TRN2 NeuronCores.

Strategy: expert-parallel sharding. The 8 experts are axis-aligned octants of
[-1,1]^3 (GRID=(2,2,2)); core e receives exactly the points routed to expert e
(the all-to-all dispatch happens on the host as part of sharding), runs a dense
single-expert 3->64->64->1 ReLU MLP over its (padded) shard, and the host
inverse-permutes the outputs.

Device mapping per core, per "round" of 4096 points (8 tiles x 512):
  L1 (K=3->M=64):  8 concurrent PE sub-array matmuls at tile_position (32i, 64j)
  L2 (K=64->M=64): 2 waves of 4 concurrent quadrant matmuls
  L3 (K=64->M=1):  8 concurrent matmuls writing y back into the (already
                   evacuated) banks 0-1 of the h2 PSUM tile
  PSUM->SBUF relu+bias evacuations are split between VectorE and ScalarE
  (bank-aligned splits so the two engines never touch the same PSUM bank).
"""

import ml_dtypes
import numpy as np

import concourse.bass as bass
import concourse.tile as tile
from concourse import bacc, mybir
from concourse.bass_utils import run_bass_kernel_spmd

F32 = mybir.dt.float32
BF16 = mybir.dt.bfloat16

N_CORES = 8
GRID = (2, 2, 2)
H = 64
F = 512              # points per tile (one PSUM-bank free dim, fp32)
TILES_PER_ROUND = 8
PTS_PER_ROUND = TILES_PER_ROUND * F  # 4096

# tile t -> (i, j) for L1/L2 input side, (a, b) for L2 output / L3 input side
def _tmap(t):
    i, j = t % 4, t // 4
    a, b = i % 2, j + 2 * (i // 2)
    return i, j, a, b


_PROGRAM_CACHE = {}
LAST_RESULTS = None  # BassKernelResults of the last run (for test harness)
LAST_IN_MAPS = None  # per-core input dicts of the last run (for test harness)
LAST_NC = None       # compiled program of the last run (for test harness)


def _build_program(nr, loop_n=None, stage="full"):
    """Build the SPMD program. loop_n (bench only): repeat the whole body
    loop_n times in a hardware For_i so device time can be measured through
    the noisy axon dispatch path by differencing two loop counts."""
    nc = bacc.Bacc(
        "TRN2",
        target_bir_lowering=False,
        debug=False,
        num_devices=N_CORES,
    )
    xT = nc.dram_tensor("xT", [nr, 4, 6, 512], BF16, kind="ExternalInput")
    w1 = nc.dram_tensor("w1", [128, 128], BF16, kind="ExternalInput")
    w2 = nc.dram_tensor("w2", [128, 128], BF16, kind="ExternalInput")
    w3 = nc.dram_tensor("w3", [128, 1], BF16, kind="ExternalInput")
    b1 = nc.dram_tensor("b1", [128, 1], F32, kind="ExternalInput")
    b2 = nc.dram_tensor("b2", [128, 1], F32, kind="ExternalInput")
    b3 = nc.dram_tensor("b3", [128, 1], F32, kind="ExternalInput")
    yO = nc.dram_tensor("y", [nr, 4, 1024], F32, kind="ExternalOutput")

    RELU = mybir.ActivationFunctionType.Relu
    ADD = mybir.AluOpType.add
    MAX = mybir.AluOpType.max

    with tile.TileContext(nc) as tc:
        with (
            tc.tile_pool(name="const", bufs=1) as const,
            tc.tile_pool(name="xin", bufs=3) as xin,
            tc.tile_pool(name="h1p", bufs=2) as h1pool,
            tc.tile_pool(name="h2p", bufs=2) as h2pool,
            tc.tile_pool(name="yout", bufs=3) as yout,
            tc.tile_pool(name="ps", bufs=4, space="PSUM") as ps,
        ):
            w1_sb = const.tile([128, 128], BF16)
            nc.sync.dma_start(w1_sb[:], w1.ap())
            w2_sb = const.tile([128, 128], BF16)
            nc.sync.dma_start(w2_sb[:], w2.ap())
            w3_sb = const.tile([128, 1], BF16)
            nc.sync.dma_start(w3_sb[:], w3.ap())
            b1_sb = const.tile([128, 1], F32)
            nc.sync.dma_start(b1_sb[:], b1.ap())
            b2_sb = const.tile([128, 1], F32)
            nc.sync.dma_start(b2_sb[:], b2.ap())
            b3_sb = const.tile([128, 1], F32)
            nc.sync.dma_start(b3_sb[:], b3.ap())

            import contextlib
            loop_cm = (
                tc.For_i(
                    0, loop_n, 1,
                    hint_engines=(
                        mybir.EngineType.PE,
                        mybir.EngineType.DVE,
                        mybir.EngineType.Activation,
                        mybir.EngineType.SP,
                    ),
                )
                if loop_n
                else contextlib.nullcontext()
            )
            with loop_cm:
              for r in range(nr):
                  # pair p holds point-tiles t=2p (slot s=0) and t=2p+1 (s=1),
                  # stacked block-diagonally: x rows 3s+c, h rows 64s+j
                  x_sb = xin.tile([128, 512], BF16)
                  for p in range(4):
                      nc.sync.dma_start(
                          x_sb[32 * p : 32 * p + 6, :], xT.ap()[r, p]
                      )

                  # ---- L1: 4 row-tiled block-diag matmuls (concurrent) ----
                  ph1a = ps.tile([128, 1024], F32, tag="hps")  # pairs 0,1
                  ph1b = ps.tile([128, 1024], F32, tag="hps")  # pairs 2,3
                  for p in range(4):
                      dst = ph1a if p < 2 else ph1b
                      nc.tensor.matmul(
                          dst[:, 512 * (p % 2) : 512 * (p % 2) + 512],
                          w1_sb[32 * p : 32 * p + 6, :],
                          x_sb[32 * p : 32 * p + 6, :],
                          start=True,
                          stop=True,
                          tile_position=(32 * p, 0),
                      )
                  # relu + bias evac: DVE takes the a-half, ACT the b-half
                  h1r = h1pool.tile([128, 2048], BF16)
                  nc.vector.tensor_scalar(
                      h1r[:, 0:1024], ph1a[:, 0:1024], b1_sb[:, 0:1], 0.0, ADD, MAX
                  )
                  nc.scalar.activation(
                      h1r[:, 1024:2048], ph1b[:, 0:1024], RELU, bias=b1_sb[:, 0:1]
                  )

                  if stage == "l1":
                      nc.sync.dma_start(
                          yO.ap()[r, 0], h1r[0:1, :].bitcast(F32)
                      )
                      continue

                  # ---- L2: 4 full-array block-diag matmuls ----
                  ph2a = ps.tile([128, 1024], F32, tag="hps")  # pairs 0,1
                  ph2b = ps.tile([128, 1024], F32, tag="hps")  # pairs 2,3
                  for p in range(4):
                      dst = ph2a if p < 2 else ph2b
                      nc.tensor.matmul(
                          dst[:, 512 * (p % 2) : 512 * (p % 2) + 512],
                          w2_sb[:, :],
                          h1r[:, 512 * p : 512 * p + 512],
                          start=True,
                          stop=True,
                          tile_position=(0, 0),
                      )
                  h2r = h2pool.tile([128, 2048], BF16)
                  nc.vector.tensor_scalar(
                      h2r[:, 0:1024], ph2a[:, 0:1024], b2_sb[:, 0:1], 0.0, ADD, MAX
                  )
                  nc.scalar.activation(
                      h2r[:, 1024:2048], ph2b[:, 0:1024], RELU, bias=b2_sb[:, 0:1]
                  )

                  if stage == "l2":
                      nc.sync.dma_start(
                          yO.ap()[r, 0], h2r[0:1, :].bitcast(F32)
                      )
                      continue

                  # ---- L3: 8 tiny matmuls into ph2a (already evacuated) ----
                  for p in range(4):
                      for s in range(2):
                          nc.tensor.matmul(
                              ph2a[32 * p : 32 * p + 1, 512 * s : 512 * s + 512],
                              w3_sb[64 * s : 64 * s + 64, 0:1],
                              h2r[64 * s : 64 * s + 64, 512 * p : 512 * p + 512],
                              start=True,
                              stop=True,
                              tile_position=(64 * s, 32 * p),
                          )
                  y_sb = yout.tile([128, 1024], F32)
                  nc.scalar.activation(
                      y_sb[:], ph2a[:, 0:1024],
                      mybir.ActivationFunctionType.Identity, bias=b3_sb[:, 0:1]
                  )
                  for c in range(4):
                      nc.sync.dma_start(
                          yO.ap()[r, c], y_sb[32 * c : 32 * c + 1, :]
                      )

    nc.compile()
    return nc


def kernel(x, extents_min, extents_max, W1, b1, W2, b2, W3, b3):
    global LAST_RESULTS
    x = np.ascontiguousarray(np.asarray(x, dtype=np.float32))
    extents_min = np.asarray(extents_min, dtype=np.float32)
    extents_max = np.asarray(extents_max, dtype=np.float32)
    W1 = np.asarray(W1, dtype=np.float32)
    b1 = np.asarray(b1, dtype=np.float32)
    W2 = np.asarray(W2, dtype=np.float32)
    b2 = np.asarray(b2, dtype=np.float32)
    W3 = np.asarray(W3, dtype=np.float32)
    b3 = np.asarray(b3, dtype=np.float32)

    n_pts = x.shape[0]
    E = W1.shape[0]
    assert E == N_CORES

    # --- routing (identical fp32 math to the reference) ---
    gvec = np.asarray(GRID, dtype=np.float32)
    u = np.clip((x + np.float32(1.0)) * np.float32(0.5), 0.0, 0.99)
    gi = (u * gvec).astype(np.int32)
    idx = gi[:, 0] + gi[:, 1] * GRID[0] + gi[:, 2] * (GRID[0] * GRID[1])

    order = np.argsort(idx, kind="stable")
    counts = np.bincount(idx, minlength=E)
    starts = np.concatenate([[0], np.cumsum(counts)[:-1]])
    x_sorted = x[order]

    nr = max(1, int(np.ceil(counts.max() / PTS_PER_ROUND)))
    cap = nr * PTS_PER_ROUND

    # --- fold the expert-local normalization into layer-1 weights ---
    # xn = s*x + t, s = 2/(emax-emin), t = -2*emin/(emax-emin) - 1
    span = extents_max - extents_min          # [E, 3]
    s = 2.0 / span
    tvec = -2.0 * extents_min / span - 1.0
    # h1_pre = x @ W1e' + b1e',  W1e' = diag(s) @ W1e, b1e' = b1e + t @ W1e
    W1p = W1 * s[:, :, None]                  # [E, 3, H]
    b1p = b1 + np.einsum("ec,ech->eh", tvec, W1)

    in_maps = []
    for e in range(E):
        xe = np.zeros((cap, 3), dtype=np.float32)
        xe[: counts[e]] = x_sorted[starts[e] : starts[e] + counts[e]]
        # xT[r, p, 3s+c, n] = xe[r*4096 + (2p+s)*512 + n, c]
        xt = (
            xe.reshape(nr, 4, 2, 512, 3)      # r, p, s, n, c
            .transpose(0, 1, 2, 4, 3)         # r, p, s, c, n
            .reshape(nr, 4, 6, 512)
            .astype(ml_dtypes.bfloat16)
        )
        # w1: 4 row strips (one per pair), each the [6,128] block-diag of W1'
        w1e = W1p[e].astype(ml_dtypes.bfloat16)
        w1_full = np.zeros((128, 128), dtype=ml_dtypes.bfloat16)
        for p in range(4):
            w1_full[32 * p : 32 * p + 3, 0:64] = w1e
            w1_full[32 * p + 3 : 32 * p + 6, 64:128] = w1e
        # w2: [128,128] block-diag of W2
        w2_full = np.zeros((128, 128), dtype=ml_dtypes.bfloat16)
        w2_full[0:64, 0:64] = W2[e].astype(ml_dtypes.bfloat16)
        w2_full[64:128, 64:128] = W2[e].astype(ml_dtypes.bfloat16)
        w3_full = np.concatenate([W3[e], W3[e]], axis=0).astype(ml_dtypes.bfloat16)
        b1_full = np.tile(b1p[e], 2)[:, None].astype(np.float32)
        b2_full = np.tile(b2[e], 2)[:, None].astype(np.float32)
        b3_full = np.full((128, 1), b3[e, 0], dtype=np.float32)
        in_maps.append(
            {
                "xT": np.ascontiguousarray(xt),
                "w1": w1_full,
                "w2": w2_full,
                "w3": w3_full,
                "b1": b1_full,
                "b2": b2_full,
                "b3": b3_full,
            }
        )

    if nr not in _PROGRAM_CACHE:
        _PROGRAM_CACHE[nr] = _build_program(nr)
    nc = _PROGRAM_CACHE[nr]

    res = run_bass_kernel_spmd(nc, in_maps, core_ids=list(range(N_CORES)))
    global LAST_IN_MAPS, LAST_NC
    LAST_RESULTS = res
    LAST_IN_MAPS = in_maps
    LAST_NC = nc

    # --- unshard: y_dev[r, p, 512s+n] -> point r*4096 + (2p+s)*512 + n ---
    y_sorted = np.empty(n_pts, dtype=np.float32)
    for e in range(E):
        ye = res.results[e]["y"].reshape(cap)
        y_sorted[starts[e] : starts[e] + counts[e]] = ye[: counts[e]]

    y_full = np.empty(n_pts, dtype=np.float32)
    y_full[order] = y_sorted
    return y_full[:, None]



# revision 3
# speedup vs baseline: 1.1186x; 1.1186x over previous
"""Ensemble-SRN MoE routing kernel for 8 TRN2 NeuronCores.

Strategy: expert-parallel sharding. The 8 experts are axis-aligned octants of
[-1,1]^3 (GRID=(2,2,2)); core e receives exactly the points routed to expert e
(the all-to-all dispatch happens on the host as part of sharding), runs a dense
single-expert 3->64->64->1 ReLU MLP over its (padded) shard, and the host
inverse-permutes the outputs.

v2 design (vs the first working version):
  - All x coordinates are preloaded into SBUF once (4 large DMAs) and all y
    stays resident in SBUF until 4 final DMAs; per-round DMA traffic is zero.
    (The HWDGE per-dma_start cost ~625ns made per-round DMAs the top
    bottleneck.)
  - Point shard processed as `fr` full rounds of 4096 points (4 pairs x 1024)
    plus a tail of `tp` pairs, sized from the actual max shard count.
  - L1: 4 strip matmuls [6,128]x[6,512] at tile_position (32p, 0).
  - L2: 4 full-array block-diag matmuls (2 experts-copies of W2).
  - L3: 4 matmuls with M=2 block-diag w3 ([128,2]) at tile_position (0,32p):
    y for pair p lands at PSUM partitions {32p, 32p+1} of a 1-bank tile.
  - Evac split DVE/ACT: DVE takes ph1a/ph2a (tensor_scalar add-bias+relu),
    ACT takes ph1b/ph2b + the y evac (activation with bias).
  - PSUM: shared 3-slot pool of [128,1024]f32 (2 banks each) for h1/h2 +
    2-slot pool of [128,512]f32 for y = 8 banks, allowing cross-round overlap.
"""

import ml_dtypes
import numpy as np

import concourse.bass as bass
import concourse.tile as tile
from concourse import bacc, mybir
from concourse.bass_utils import run_bass_kernel_spmd

F32 = mybir.dt.float32
BF16 = mybir.dt.bfloat16

N_CORES = 8
GRID = (2, 2, 2)
H = 64
F = 512              # points per tile (one PSUM-bank free dim, fp32)

_PROGRAM_CACHE = {}
LAST_RESULTS = None  # BassKernelResults of the last run (for test harness)
LAST_IN_MAPS = None  # per-core input dicts of the last run (for test harness)
LAST_NC = None       # compiled program of the last run (for test harness)
LAST_CFG = None      # (fr, tp) of the last run (for test harness)


def _build_program(cfg, loop_n=None, stage="full"):
    """Build the SPMD program for cfg=(full_rounds, tail_pairs). loop_n
    (bench only): repeat the whole body loop_n times in a hardware For_i so
    device time can be measured through the noisy axon dispatch path by
    differencing two loop counts."""
    fr, tp = cfg
    nr = fr + (1 if tp else 0)          # column blocks in x/y SBUF tiles
    nc = bacc.Bacc(
        "TRN2",
        target_bir_lowering=False,
        debug=False,
        num_devices=N_CORES,
    )
    xT = nc.dram_tensor("xT", [4, 6, nr * F], BF16, kind="ExternalInput")
    w1 = nc.dram_tensor("w1", [128, 128], BF16, kind="ExternalInput")
    w2 = nc.dram_tensor("w2", [128, 128], BF16, kind="ExternalInput")
    w3 = nc.dram_tensor("w3", [128, 2], BF16, kind="ExternalInput")
    b1 = nc.dram_tensor("b1", [128, 1], F32, kind="ExternalInput")
    b2 = nc.dram_tensor("b2", [128, 1], F32, kind="ExternalInput")
    b3 = nc.dram_tensor("b3", [128, 1], F32, kind="ExternalInput")
    yO = nc.dram_tensor("y", [4, 2, nr * F], F32, kind="ExternalOutput")

    RELU = mybir.ActivationFunctionType.Relu
    IDENT = mybir.ActivationFunctionType.Identity
    ADD = mybir.AluOpType.add
    MAX = mybir.AluOpType.max

    with tile.TileContext(nc) as tc:
        with (
            tc.tile_pool(name="const", bufs=1) as const,
            tc.tile_pool(name="xall", bufs=1) as xall,
            tc.tile_pool(name="yall", bufs=1) as yall,
            tc.tile_pool(name="h1p", bufs=2) as h1pool,
            tc.tile_pool(name="h2p", bufs=2) as h2pool,
            tc.tile_pool(name="ps", bufs=3, space="PSUM") as ps,
            tc.tile_pool(name="yp", bufs=2, space="PSUM") as yp,
        ):
            w1_sb = const.tile([128, 128], BF16)
            nc.sync.dma_start(w1_sb[:], w1.ap())
            w2_sb = const.tile([128, 128], BF16)
            nc.sync.dma_start(w2_sb[:], w2.ap())
            w3_sb = const.tile([128, 2], BF16)
            nc.sync.dma_start(w3_sb[:], w3.ap())
            b1_sb = const.tile([128, 1], F32)
            nc.sync.dma_start(b1_sb[:], b1.ap())
            b2_sb = const.tile([128, 1], F32)
            nc.sync.dma_start(b2_sb[:], b2.ap())
            b3_sb = const.tile([128, 1], F32)
            nc.sync.dma_start(b3_sb[:], b3.ap())

            import contextlib
            loop_cm = (
                tc.For_i(
                    0, loop_n, 1,
                    hint_engines=(
                        mybir.EngineType.PE,
                        mybir.EngineType.DVE,
                        mybir.EngineType.Activation,
                        mybir.EngineType.SP,
                    ),
                )
                if loop_n
                else contextlib.nullcontext()
            )
            with loop_cm:
                x_sb = xall.tile([128, nr * F], BF16)
                for p in range(4):
                    nc.sync.dma_start(
                        x_sb[32 * p : 32 * p + 6, :], xT.ap()[p]
                    )
                y_sb = yall.tile([128, nr * F], F32)

                def round_body(r, pairs):
                    """One round of `pairs` pairs (pairs=4 for full rounds)."""
                    c0 = r * F  # column base in x_sb / y_sb
                    # ---- L1 ----
                    if pairs == 4:
                        ph1a = ps.tile([128, 1024], F32, tag="hps")
                        ph1b = ps.tile([128, 1024], F32, tag="hps")
                        ph1 = [ph1a, ph1b]
                    else:
                        ph1t = ps.tile([128, F * pairs], F32, tag="hps")
                        ph1 = [ph1t]
                    for p in range(pairs):
                        nc.tensor.matmul(
                            ph1[p // 2][:, F * (p % 2) : F * (p % 2) + F],
                            w1_sb[32 * p : 32 * p + 6, :],
                            x_sb[32 * p : 32 * p + 6, c0 : c0 + F],
                            start=True,
                            stop=True,
                            tile_position=(32 * p, 0),
                        )
                    # relu + bias evac: DVE takes the a-half, ACT the b-half
                    h1r = h1pool.tile([128, F * pairs], BF16, tag="h1r")
                    if pairs == 4:
                        nc.vector.tensor_scalar(
                            h1r[:, 0:1024], ph1[0][:], b1_sb[:, 0:1],
                            0.0, ADD, MAX,
                        )
                        nc.scalar.activation(
                            h1r[:, 1024:2048], ph1[1][:], RELU,
                            bias=b1_sb[:, 0:1],
                        )
                    else:
                        nc.vector.tensor_scalar(
                            h1r[:], ph1[0][:], b1_sb[:, 0:1], 0.0, ADD, MAX,
                        )

                    # ---- L2 ----
                    if pairs == 4:
                        ph2a = ps.tile([128, 1024], F32, tag="hps")
                        ph2b = ps.tile([128, 1024], F32, tag="hps")
                        ph2 = [ph2a, ph2b]
                    else:
                        ph2t = ps.tile([128, F * pairs], F32, tag="hps")
                        ph2 = [ph2t]
                    for p in range(pairs):
                        nc.tensor.matmul(
                            ph2[p // 2][:, F * (p % 2) : F * (p % 2) + F],
                            w2_sb[:, :],
                            h1r[:, F * p : F * p + F],
                            start=True,
                            stop=True,
                            tile_position=(0, 0),
                        )
                    h2r = h2pool.tile([128, F * pairs], BF16, tag="h2r")
                    if pairs == 4:
                        nc.vector.tensor_scalar(
                            h2r[:, 0:1024], ph2[0][:], b2_sb[:, 0:1],
                            0.0, ADD, MAX,
                        )
                        nc.scalar.activation(
                            h2r[:, 1024:2048], ph2[1][:], RELU,
                            bias=b2_sb[:, 0:1],
                        )
                    else:
                        nc.vector.tensor_scalar(
                            h2r[:], ph2[0][:], b2_sb[:, 0:1], 0.0, ADD, MAX,
                        )

                    # ---- L3: M=2 block-diag w3, y -> partitions {32p,32p+1}
                    yps = yp.tile([128, F], F32, tag="yps")
                    for p in range(pairs):
                        nc.tensor.matmul(
                            yps[32 * p : 32 * p + 2, :],
                            w3_sb[:, 0:2],
                            h2r[:, F * p : F * p + F],
                            start=True,
                            stop=True,
                            tile_position=(0, 32 * p),
                        )
                    nc.scalar.activation(
                        y_sb[:, c0 : c0 + F], yps[:], IDENT, bias=b3_sb[:, 0:1]
                    )

                for r in range(fr):
                    round_body(r, 4)
                if tp:
                    round_body(fr, tp)

                for g in range(4):
                    nc.sync.dma_start(
                        yO.ap()[g], y_sb[32 * g : 32 * g + 2, :]
                    )

    nc.compile()
    return nc


def kernel(x, extents_min, extents_max, W1, b1, W2, b2, W3, b3):
    global LAST_RESULTS, LAST_IN_MAPS, LAST_NC, LAST_CFG
    x = np.ascontiguousarray(np.asarray(x, dtype=np.float32))
    extents_min = np.asarray(extents_min, dtype=np.float32)
    extents_max = np.asarray(extents_max, dtype=np.float32)
    W1 = np.asarray(W1, dtype=np.float32)
    b1 = np.asarray(b1, dtype=np.float32)
    W2 = np.asarray(W2, dtype=np.float32)
    b2 = np.asarray(b2, dtype=np.float32)
    W3 = np.asarray(W3, dtype=np.float32)
    b3 = np.asarray(b3, dtype=np.float32)

    n_pts = x.shape[0]
    E = W1.shape[0]
    assert E == N_CORES

    # --- routing (identical fp32 math to the reference) ---
    gvec = np.asarray(GRID, dtype=np.float32)
    u = np.clip((x + np.float32(1.0)) * np.float32(0.5), 0.0, 0.99)
    gi = (u * gvec).astype(np.int32)
    idx = gi[:, 0] + gi[:, 1] * GRID[0] + gi[:, 2] * (GRID[0] * GRID[1])

    order = np.argsort(idx, kind="stable")
    counts = np.bincount(idx, minlength=E)
    starts = np.concatenate([[0], np.cumsum(counts)[:-1]])
    x_sorted = x[order]

    maxc = int(counts.max())
    fr = maxc // 4096
    tp = -(-(maxc - fr * 4096) // 1024)  # ceil
    if tp == 4:
        fr, tp = fr + 1, 0
    if fr == 0 and tp == 0:
        tp = 1
    cap = fr * 4096 + tp * 1024
    npairs = fr * 4 + tp
    nr = fr + (1 if tp else 0)

    # --- fold the expert-local normalization into layer-1 weights ---
    # xn = s*x + t, s = 2/(emax-emin), t = -2*emin/(emax-emin) - 1
    span = extents_max - extents_min          # [E, 3]
    s = 2.0 / span
    tvec = -2.0 * extents_min / span - 1.0
    W1p = W1 * s[:, :, None]                  # [E, 3, H]
    b1p = b1 + np.einsum("ec,ech->eh", tvec, W1)

    in_maps = []
    for e in range(E):
        xe = np.zeros((cap, 3), dtype=np.float32)
        xe[: counts[e]] = x_sorted[starts[e] : starts[e] + counts[e]]
        # xT[p, 3s+c, k*512+n] = xe[(4k+p)*1024 + s*512 + n, c]
        xq = xe.reshape(npairs, 2, F, 3)
        xt = np.zeros((4, 6, nr * F), dtype=ml_dtypes.bfloat16)
        for p in range(4):
            nq = fr + (1 if p < tp else 0)
            blk = xq[p::4]                     # [nq, 2, 512, 3]
            assert blk.shape[0] == nq
            # rows 3s+c, cols k*512+n
            xt[p, :, : nq * F] = (
                blk.transpose(1, 3, 0, 2).reshape(6, nq * F)
                .astype(ml_dtypes.bfloat16)
            )
        # w1: 4 row strips (one per pair), each the [6,128] block-diag of W1'
        w1e = W1p[e].astype(ml_dtypes.bfloat16)
        w1_full = np.zeros((128, 128), dtype=ml_dtypes.bfloat16)
        for p in range(4):
            w1_full[32 * p : 32 * p + 3, 0:64] = w1e
            w1_full[32 * p + 3 : 32 * p + 6, 64:128] = w1e
        # w2: [128,128] block-diag of W2
        w2_full = np.zeros((128, 128), dtype=ml_dtypes.bfloat16)
        w2_full[0:64, 0:64] = W2[e].astype(ml_dtypes.bfloat16)
        w2_full[64:128, 64:128] = W2[e].astype(ml_dtypes.bfloat16)
        # w3: [128,2] block-diag
        w3_full = np.zeros((128, 2), dtype=ml_dtypes.bfloat16)
        w3_full[0:64, 0] = W3[e, :, 0].astype(ml_dtypes.bfloat16)
        w3_full[64:128, 1] = W3[e, :, 0].astype(ml_dtypes.bfloat16)
        b1_full = np.tile(b1p[e], 2)[:, None].astype(np.float32)
        b2_full = np.tile(b2[e], 2)[:, None].astype(np.float32)
        b3_full = np.full((128, 1), b3[e, 0], dtype=np.float32)
        in_maps.append(
            {
                "xT": np.ascontiguousarray(xt),
                "w1": w1_full,
                "w2": w2_full,
                "w3": w3_full,
                "b1": b1_full,
                "b2": b2_full,
                "b3": b3_full,
            }
        )

    cfg = (fr, tp)
    if cfg not in _PROGRAM_CACHE:
        _PROGRAM_CACHE[cfg] = _build_program(cfg)
    nc = _PROGRAM_CACHE[cfg]

    res = run_bass_kernel_spmd(nc, in_maps, core_ids=list(range(N_CORES)))
    LAST_RESULTS = res
    LAST_IN_MAPS = in_maps
    LAST_NC = nc
    LAST_CFG = cfg

    # --- unshard: y_dev[p, s, k*512+n] -> point (4k+p)*1024 + s*512 + n ---
    y_sorted = np.empty(n_pts, dtype=np.float32)
    for e in range(E):
        ydev = res.results[e]["y"]             # [4, 2, nr*512]
        ye = np.empty((npairs, 1024), dtype=np.float32)
        for p in range(4):
            nq = fr + (1 if p < tp else 0)
            blk = ydev[p, :, : nq * F].reshape(2, nq, F)
            ye[p::4] = blk.transpose(1, 0, 2).reshape(nq, 1024)
        yflat = ye.reshape(cap)
        y_sorted[starts[e] : starts[e] + counts[e]] = yflat[: counts[e]]

    y_full = np.empty(n_pts, dtype=np.float32)
    y_full[order] = y_sorted
    return y_full[:, None]


# revision 18
# speedup vs baseline: 1.3344x; 1.1929x over previous
"""Ensemble-SRN MoE routing kernel for 8 TRN2 NeuronCores.

Strategy: expert-parallel sharding. The 8 experts are axis-aligned octants of
[-1,1]^3 (GRID=(2,2,2)); core e receives exactly the points routed to expert e
(the all-to-all dispatch happens on the host as part of sharding), runs a dense
single-expert 3->64->64->1 ReLU MLP over its (padded) shard, and the host
inverse-permutes the outputs.

v2 design (vs the first working version):
  - All x coordinates are preloaded into SBUF once (4 large DMAs) and all y
    stays resident in SBUF until 4 final DMAs; per-round DMA traffic is zero.
    (The HWDGE per-dma_start cost ~625ns made per-round DMAs the top
    bottleneck.)
  - Point shard processed as `fr` full rounds of 4096 points (4 pairs x 1024)
    plus a tail of `tp` pairs, sized from the actual max shard count.
  - L1: 4 strip matmuls [6,128]x[6,512] at tile_position (32p, 0).
  - L2: 4 full-array block-diag matmuls (2 experts-copies of W2).
  - L3: 4 matmuls with M=2 block-diag w3 ([128,2]) at tile_position (0,32p):
    y for pair p lands at PSUM partitions {32p, 32p+1} of a 1-bank tile.
  - Evac split DVE/ACT: DVE takes ph1a/ph2a (tensor_scalar add-bias+relu),
    ACT takes ph1b/ph2b + the y evac (activation with bias).
  - PSUM: shared 3-slot pool of [128,1024]f32 (2 banks each) for h1/h2 +
    2-slot pool of [128,512]f32 for y = 8 banks, allowing cross-round overlap.
"""

import ml_dtypes
import numpy as np

import concourse.bass as bass
import concourse.tile as tile
from concourse import bacc, mybir
from concourse.bass_utils import run_bass_kernel_spmd

F32 = mybir.dt.float32
BF16 = mybir.dt.bfloat16
U8 = mybir.dt.uint8

N_CORES = 8
GRID = (2, 2, 2)
H = 64
F = 512              # points per tile (one PSUM-bank free dim, fp32)

_PROGRAM_CACHE = {}
LAST_RESULTS = None  # BassKernelResults of the last run (for test harness)
LAST_IN_MAPS = None  # per-core input dicts of the last run (for test harness)
LAST_NC = None       # compiled program of the last run (for test harness)
LAST_CFG = None      # (fr, tp) of the last run (for test harness)


def _build_program(cfg, loop_n=None, stage="full"):
    """Build the SPMD program for cfg=(full_rounds, tail_pairs). loop_n
    (bench only): repeat the whole body loop_n times in a hardware For_i so
    device time can be measured through the noisy axon dispatch path by
    differencing two loop counts."""
    fr, tp = cfg
    nr = fr + (1 if tp else 0)          # column blocks in x/y SBUF tiles
    nc = bacc.Bacc(
        "TRN2",
        target_bir_lowering=False,
        debug=False,
        num_devices=N_CORES,
    )
    xT = nc.dram_tensor("xT", [4, 6, nr * F], BF16, kind="ExternalInput")
    # packed constants, one DMA: w1[0:256) w2[256:512) w3[512:516) b1[516:520)
    # b2[520:524) b3[524:528) bytes per partition
    cst = nc.dram_tensor("cst", [128, 528], U8, kind="ExternalInput")
    yO = nc.dram_tensor("y", [4, 2, nr * F], F32, kind="ExternalOutput")

    RELU = mybir.ActivationFunctionType.Relu
    IDENT = mybir.ActivationFunctionType.Identity
    ADD = mybir.AluOpType.add
    MAX = mybir.AluOpType.max

    with tile.TileContext(nc) as tc:
        with (
            tc.tile_pool(name="const", bufs=1) as const,
            tc.tile_pool(name="xall", bufs=1) as xall,
            tc.tile_pool(name="yall", bufs=1) as yall,
            tc.tile_pool(name="h1p", bufs=2) as h1pool,
            tc.tile_pool(name="h2p", bufs=2) as h2pool,
            tc.tile_pool(name="ps1", bufs=2, space="PSUM") as ps1,
            tc.tile_pool(name="ps2", bufs=2, space="PSUM") as ps2,
        ):
            c_sb = const.tile([128, 528], U8)
            nc.sync.dma_start(c_sb[:], cst.ap())
            w1_sb = c_sb[:, 0:256].bitcast(BF16)
            w2_sb = c_sb[:, 256:512].bitcast(BF16)
            w3_sb = c_sb[:, 512:516].bitcast(BF16)
            b1_sb = c_sb[:, 516:520].bitcast(F32)
            b2_sb = c_sb[:, 520:524].bitcast(F32)
            b3_sb = c_sb[:, 524:528].bitcast(F32)

            import contextlib
            loop_cm = (
                tc.For_i(
                    0, loop_n, 1,
                    hint_engines=(
                        mybir.EngineType.PE,
                        mybir.EngineType.DVE,
                        mybir.EngineType.Activation,
                        mybir.EngineType.SP,
                    ),
                )
                if loop_n
                else contextlib.nullcontext()
            )
            with loop_cm:
                x_sb = xall.tile([128, nr * F], BF16)
                # NOTE: multi-partition-dim APs (rearrange "(g sub) f") are
                # silently wrong through the DMA path on HW - use 4 plain
                # contiguous-partition DMAs.
                for p in range(4):
                    nc.sync.dma_start(
                        x_sb[32 * p : 32 * p + 6, :], xT.ap()[p]
                    )
                y_sb = yall.tile([128, nr * F], F32)

                # constant h tiles for stage="pe" (decouple MMs from evacs)
                if stage == "pe":
                    h1c = const.tile([128, 2048], BF16)
                    nc.vector.memset(h1c[:], 1.0)
                    h2c = const.tile([128, 2048], BF16)
                    nc.vector.memset(h2c[:], 1.0)

                rounds = [(r, 4) for r in range(fr)]
                if tp:
                    rounds.append((fr, tp))
                n = len(rounds)
                S = [dict() for _ in range(n)]

                def emit_L1(i):
                    r, pairs = rounds[i]
                    c0 = r * F
                    if pairs == 4:
                        ph1a = ps1.tile([128, 1024], F32, tag="hp1")
                        ph1b = ps1.tile([128, 1024], F32, tag="hp1")
                        ph1 = [ph1a, ph1b]
                    else:
                        ph1t = ps1.tile([128, F * pairs], F32, tag="hp1")
                        ph1 = [ph1t]
                    S[i].update(ph1=ph1, c0=c0, pairs=pairs)
                    if stage != "evac":
                        for p in range(pairs):
                            nc.tensor.matmul(
                                ph1[p // 2][:, F * (p % 2) : F * (p % 2) + F],
                                w1_sb[32 * p : 32 * p + 6, :],
                                x_sb[32 * p : 32 * p + 6, c0 : c0 + F],
                                start=True,
                                stop=True,
                                tile_position=(32 * p, 0),
                            )

                def emit_evac1(i):  # DVE: bias+relu for both L1 halves
                    pairs = S[i]["pairs"]
                    ph1 = S[i]["ph1"]
                    h1r = h1pool.tile([128, F * pairs], BF16, tag="h1r")
                    S[i]["h1r"] = h1r
                    if stage == "pe":
                        return
                    if pairs == 4:
                        nc.vector.tensor_scalar(
                            h1r[:, 0:1024], ph1[0][:], b1_sb[:, 0:1],
                            0.0, ADD, MAX,
                        )
                        nc.vector.tensor_scalar(
                            h1r[:, 1024:2048], ph1[1][:], b1_sb[:, 0:1],
                            0.0, ADD, MAX,
                        )
                    else:
                        nc.vector.tensor_scalar(
                            h1r[:], ph1[0][:], b1_sb[:, 0:1], 0.0, ADD, MAX,
                        )

                def emit_L2(i):
                    pairs = S[i]["pairs"]
                    h1in = h1c if stage == "pe" else S[i]["h1r"]
                    if pairs == 4:
                        ph2a = ps2.tile([128, 1024], F32, tag="hp2")
                        ph2b = ps2.tile([128, 1024], F32, tag="hp2")
                        ph2 = [ph2a, ph2b]
                    else:
                        ph2t = ps2.tile([128, F * pairs], F32, tag="hp2")
                        ph2 = [ph2t]
                    S[i]["ph2"] = ph2
                    if stage != "evac":
                        for p in range(pairs):
                            nc.tensor.matmul(
                                ph2[p // 2][:, F * (p % 2) : F * (p % 2) + F],
                                w2_sb[:, :],
                                h1in[:, F * p : F * p + F],
                                start=True,
                                stop=True,
                                tile_position=(0, 0),
                            )

                def emit_evac2(i):  # ACT: bias+relu for both L2 halves
                    pairs = S[i]["pairs"]
                    ph2 = S[i]["ph2"]
                    h2r = h2pool.tile([128, F * pairs], BF16, tag="h2r")
                    S[i]["h2r"] = h2r
                    if stage == "pe":
                        return
                    if pairs == 4:
                        nc.scalar.activation(
                            h2r[:, 0:1024], ph2[0][:], RELU,
                            bias=b2_sb[:, 0:1],
                        )
                        nc.scalar.activation(
                            h2r[:, 1024:2048], ph2[1][:], RELU,
                            bias=b2_sb[:, 0:1],
                        )
                    else:
                        nc.vector.tensor_scalar(
                            h2r[:], ph2[0][:], b2_sb[:, 0:1], 0.0, ADD, MAX,
                        )

                def emit_L3(i):
                    pairs = S[i]["pairs"]
                    h2in = h2c if stage == "pe" else S[i]["h2r"]
                    yt = S[i]["ph2"][-1]
                    if stage != "evac":
                        for p in range(pairs):
                            nc.tensor.matmul(
                                yt[32 * p : 32 * p + 2, 0:F],
                                w3_sb[:, 0:2],
                                h2in[:, F * p : F * p + F],
                                start=True,
                                stop=True,
                                tile_position=(0, 32 * p),
                            )

                def emit_y(i):  # ACT: bias-add copy of y psum into y_sb
                    if stage == "pe":
                        return
                    yt = S[i]["ph2"][-1]
                    c0 = S[i]["c0"]
                    nc.scalar.activation(
                        y_sb[:, c0 : c0 + F], yt[:, 0:F],
                        IDENT, bias=b3_sb[:, 0:1]
                    )

                h = fr // 2  # first y-DMA wave covers rounds 0..h-1
                # software-pipelined emission: each engine's queue is ordered
                # so the head is (expected) always ready:
                #   PE : L1(0), L2(0), L1(1), L3(0), L2(1), L1(2), L3(1), ...
                #   DVE: L1a(i), L1b(i)
                #   ACT: y(i-1), L2a(i), L2b(i)
                emit_L1(0)
                for i in range(n):
                    emit_evac1(i)
                    if i >= 1:
                        emit_y(i - 1)
                    if i == h and h > 0:
                        for g in range(4):
                            nc.sync.dma_start(
                                yO.ap()[g][:, 0 : h * F],
                                y_sb[32 * g : 32 * g + 2, 0 : h * F],
                            )
                    emit_L2(i)
                    if i + 1 < n:
                        emit_L1(i + 1)
                    emit_evac2(i)
                    emit_L3(i)
                emit_y(n - 1)

                for g in range(4):
                    nc.sync.dma_start(
                        yO.ap()[g][:, h * F :],
                        y_sb[32 * g : 32 * g + 2, h * F :],
                    )

    nc.compile()
    return nc


def kernel(x, extents_min, extents_max, W1, b1, W2, b2, W3, b3):
    global LAST_RESULTS, LAST_IN_MAPS, LAST_NC, LAST_CFG
    x = np.ascontiguousarray(np.asarray(x, dtype=np.float32))
    extents_min = np.asarray(extents_min, dtype=np.float32)
    extents_max = np.asarray(extents_max, dtype=np.float32)
    W1 = np.asarray(W1, dtype=np.float32)
    b1 = np.asarray(b1, dtype=np.float32)
    W2 = np.asarray(W2, dtype=np.float32)
    b2 = np.asarray(b2, dtype=np.float32)
    W3 = np.asarray(W3, dtype=np.float32)
    b3 = np.asarray(b3, dtype=np.float32)

    n_pts = x.shape[0]
    E = W1.shape[0]
    assert E == N_CORES

    # --- routing (identical fp32 math to the reference) ---
    gvec = np.asarray(GRID, dtype=np.float32)
    u = np.clip((x + np.float32(1.0)) * np.float32(0.5), 0.0, 0.99)
    gi = (u * gvec).astype(np.int32)
    idx = gi[:, 0] + gi[:, 1] * GRID[0] + gi[:, 2] * (GRID[0] * GRID[1])

    order = np.argsort(idx, kind="stable")
    counts = np.bincount(idx, minlength=E)
    starts = np.concatenate([[0], np.cumsum(counts)[:-1]])
    x_sorted = x[order]

    maxc = int(counts.max())
    fr = maxc // 4096
    tp = -(-(maxc - fr * 4096) // 1024)  # ceil
    if tp == 4:
        fr, tp = fr + 1, 0
    if fr == 0 and tp == 0:
        tp = 1
    cap = fr * 4096 + tp * 1024
    npairs = fr * 4 + tp
    nr = fr + (1 if tp else 0)

    # --- fold the expert-local normalization into layer-1 weights ---
    # xn = s*x + t, s = 2/(emax-emin), t = -2*emin/(emax-emin) - 1
    span = extents_max - extents_min          # [E, 3]
    s = 2.0 / span
    tvec = -2.0 * extents_min / span - 1.0
    W1p = W1 * s[:, :, None]                  # [E, 3, H]
    b1p = b1 + np.einsum("ec,ech->eh", tvec, W1)

    in_maps = []
    for e in range(E):
        xe = np.zeros((cap, 3), dtype=np.float32)
        xe[: counts[e]] = x_sorted[starts[e] : starts[e] + counts[e]]
        # xT[p, 3s+c, k*512+n] = xe[(4k+p)*1024 + s*512 + n, c]
        xq = xe.reshape(npairs, 2, F, 3)
        xt = np.zeros((4, 6, nr * F), dtype=ml_dtypes.bfloat16)
        for p in range(4):
            nq = fr + (1 if p < tp else 0)
            blk = xq[p::4]                     # [nq, 2, 512, 3]
            assert blk.shape[0] == nq
            # rows 3s+c, cols k*512+n
            xt[p, :, : nq * F] = (
                blk.transpose(1, 3, 0, 2).reshape(6, nq * F)
                .astype(ml_dtypes.bfloat16)
            )
        # w1: 4 row strips (one per pair), each the [6,128] block-diag of W1'
        w1e = W1p[e].astype(ml_dtypes.bfloat16)
        w1_full = np.zeros((128, 128), dtype=ml_dtypes.bfloat16)
        for p in range(4):
            w1_full[32 * p : 32 * p + 3, 0:64] = w1e
            w1_full[32 * p + 3 : 32 * p + 6, 64:128] = w1e
        # w2: [128,128] block-diag of W2
        w2_full = np.zeros((128, 128), dtype=ml_dtypes.bfloat16)
        w2_full[0:64, 0:64] = W2[e].astype(ml_dtypes.bfloat16)
        w2_full[64:128, 64:128] = W2[e].astype(ml_dtypes.bfloat16)
        # w3: [128,2] block-diag
        w3_full = np.zeros((128, 2), dtype=ml_dtypes.bfloat16)
        w3_full[0:64, 0] = W3[e, :, 0].astype(ml_dtypes.bfloat16)
        w3_full[64:128, 1] = W3[e, :, 0].astype(ml_dtypes.bfloat16)
        b1_full = np.tile(b1p[e], 2)[:, None].astype(np.float32)
        b2_full = np.tile(b2[e], 2)[:, None].astype(np.float32)
        b3_full = np.full((128, 1), b3[e, 0], dtype=np.float32)
        cst = np.concatenate(
            [
                w1_full.view(np.uint8),
                w2_full.view(np.uint8),
                w3_full.view(np.uint8),
                b1_full.view(np.uint8),
                b2_full.view(np.uint8),
                b3_full.view(np.uint8),
            ],
            axis=1,
        )
        assert cst.shape == (128, 528), cst.shape
        in_maps.append(
            {
                "xT": np.ascontiguousarray(xt),
                "cst": np.ascontiguousarray(cst),
            }
        )

    cfg = (fr, tp)
    if cfg not in _PROGRAM_CACHE:
        _PROGRAM_CACHE[cfg] = _build_program(cfg)
    nc = _PROGRAM_CACHE[cfg]

    res = run_bass_kernel_spmd(nc, in_maps, core_ids=list(range(N_CORES)))
    LAST_RESULTS = res
    LAST_IN_MAPS = in_maps
    LAST_NC = nc
    LAST_CFG = cfg

    # --- unshard: y_dev[p, s, k*512+n] -> point (4k+p)*1024 + s*512 + n ---
    y_sorted = np.empty(n_pts, dtype=np.float32)
    for e in range(E):
        ydev = res.results[e]["y"]             # [4, 2, nr*512]
        ye = np.empty((npairs, 1024), dtype=np.float32)
        for p in range(4):
            nq = fr + (1 if p < tp else 0)
            blk = ydev[p, :, : nq * F].reshape(2, nq, F)
            ye[p::4] = blk.transpose(1, 0, 2).reshape(nq, 1024)
        yflat = ye.reshape(cap)
        y_sorted[starts[e] : starts[e] + counts[e]] = yflat[: counts[e]]

    y_full = np.empty(n_pts, dtype=np.float32)
    y_full[order] = y_sorted
    return y_full[:, None]


# revision 20
# speedup vs baseline: 1.3385x; 1.0030x over previous
"""Ensemble-SRN MoE routing kernel for 8 TRN2 NeuronCores.

Strategy: expert-parallel sharding. The 8 experts are axis-aligned octants of
[-1,1]^3 (GRID=(2,2,2)); core e receives exactly the points routed to expert e
(the all-to-all dispatch happens on the host as part of sharding), runs a dense
single-expert 3->64->64->1 ReLU MLP over its (padded) shard, and the host
inverse-permutes the outputs.

Device kernel (per core, fr rounds of 4096 points = 4 pairs x 1024, plus a
tail of tp pairs sized from the actual max shard count):
  - All x coordinates are preloaded into SBUF once (4 DMAs, one per 6-row
    partition strip) and all y stays SBUF-resident until 2 waves of 4 output
    DMAs; per-round DMA traffic is zero.  (HWDGE costs ~625ns per dma_start,
    so per-round DMAs were the original top bottleneck.)
  - All weights/biases are packed into one 528-byte-per-partition uint8
    tensor, DMA'd once, and accessed through bitcast views.
  - L1 (3->64): 4 strip matmuls [6,128]x[6,512] at tile_position (32p, 0),
    two point-tiles block-diagonal per matmul.
  - L2 (64->64): 4 full-array matmuls with W2 block-diagonal twice.
  - L3 (64->1): 4 matmuls with M=2 block-diag w3 ([128,2]) at tile_position
    (0,32p); y for pair p lands at PSUM partitions {32p,32p+1} of bank 0 of
    ph2b (reused after its evacuation).
  - Evacuation split: DVE (tensor_scalar add-bias+relu) takes both L1
    halves; ACT (activation relu/identity with bias) takes both L2 halves
    plus the y evac.  Per round: DVE 2x[128,1024], ACT 2x[128,1024] +
    [128,512] - the ACT chain (~2.7us) is the steady-state critical path.
  - PSUM: ph1 pool 2x[128,1024]f32 + ph2 pool 2x[128,1024]f32 = all 8 banks;
    emission is software-pipelined (y evac of round i-1 ordered before round
    i's L2 evacs) so ACT streams without waiting on L3.
  - CAUTION: multi-partition-dim DMA APs (rearrange "(g sub) f -> g sub f")
    are silently wrong on hardware; only plain contiguous-partition slices
    are used for DMA.
"""

import ml_dtypes
import numpy as np

import concourse.bass as bass
import concourse.tile as tile
from concourse import bacc, mybir
from concourse.bass_utils import run_bass_kernel_spmd

F32 = mybir.dt.float32
BF16 = mybir.dt.bfloat16
U8 = mybir.dt.uint8

N_CORES = 8
GRID = (2, 2, 2)
H = 64
F = 512              # points per tile (one PSUM-bank free dim, fp32)

_PROGRAM_CACHE = {}
LAST_RESULTS = None  # BassKernelResults of the last run (for test harness)
LAST_IN_MAPS = None  # per-core input dicts of the last run (for test harness)
LAST_NC = None       # compiled program of the last run (for test harness)
LAST_CFG = None      # (fr, tp) of the last run (for test harness)


def _build_program(cfg, loop_n=None, stage="full"):
    """Build the SPMD program for cfg=(full_rounds, tail_pairs). loop_n
    (bench only): repeat the whole body loop_n times in a hardware For_i so
    device time can be measured through the noisy axon dispatch path by
    differencing two loop counts."""
    fr, tp = cfg
    nr = fr + (1 if tp else 0)          # column blocks in x/y SBUF tiles
    nc = bacc.Bacc(
        "TRN2",
        target_bir_lowering=False,
        debug=False,
        num_devices=N_CORES,
    )
    xT = nc.dram_tensor("xT", [4, 6, nr * F], BF16, kind="ExternalInput")
    # packed constants, one DMA: w1[0:256) w2[256:512) w3[512:516) b1[516:520)
    # b2[520:524) b3[524:528) bytes per partition
    cst = nc.dram_tensor("cst", [128, 528], U8, kind="ExternalInput")
    yO = nc.dram_tensor("y", [4, 2, nr * F], F32, kind="ExternalOutput")

    RELU = mybir.ActivationFunctionType.Relu
    IDENT = mybir.ActivationFunctionType.Identity
    ADD = mybir.AluOpType.add
    MAX = mybir.AluOpType.max

    with tile.TileContext(nc) as tc:
        with (
            tc.tile_pool(name="const", bufs=1) as const,
            tc.tile_pool(name="xall", bufs=1) as xall,
            tc.tile_pool(name="yall", bufs=1) as yall,
            tc.tile_pool(name="h1p", bufs=2) as h1pool,
            tc.tile_pool(name="h2p", bufs=2) as h2pool,
            tc.tile_pool(name="ps1", bufs=2, space="PSUM") as ps1,
            tc.tile_pool(name="ps2", bufs=2, space="PSUM") as ps2,
        ):
            c_sb = const.tile([128, 528], U8)
            nc.sync.dma_start(c_sb[:], cst.ap())
            w1_sb = c_sb[:, 0:256].bitcast(BF16)
            w2_sb = c_sb[:, 256:512].bitcast(BF16)
            w3_sb = c_sb[:, 512:516].bitcast(BF16)
            b1_sb = c_sb[:, 516:520].bitcast(F32)
            b2_sb = c_sb[:, 520:524].bitcast(F32)
            b3_sb = c_sb[:, 524:528].bitcast(F32)

            import contextlib
            loop_cm = (
                tc.For_i(
                    0, loop_n, 1,
                    hint_engines=(
                        mybir.EngineType.PE,
                        mybir.EngineType.DVE,
                        mybir.EngineType.Activation,
                        mybir.EngineType.SP,
                    ),
                )
                if loop_n
                else contextlib.nullcontext()
            )
            with loop_cm:
                x_sb = xall.tile([128, nr * F], BF16)
                # NOTE: multi-partition-dim APs (rearrange "(g sub) f") are
                # silently wrong through the DMA path on HW - use 4 plain
                # contiguous-partition DMAs.
                for p in range(4):
                    nc.sync.dma_start(
                        x_sb[32 * p : 32 * p + 6, :], xT.ap()[p]
                    )
                y_sb = yall.tile([128, nr * F], F32)

                # constant h tiles for stage="pe" (decouple MMs from evacs)
                if stage == "pe":
                    h1c = const.tile([128, 2048], BF16)
                    nc.vector.memset(h1c[:], 1.0)
                    h2c = const.tile([128, 2048], BF16)
                    nc.vector.memset(h2c[:], 1.0)

                rounds = [(r, 4) for r in range(fr)]
                if tp:
                    rounds.append((fr, tp))
                n = len(rounds)
                S = [dict() for _ in range(n)]

                def emit_L1(i):
                    r, pairs = rounds[i]
                    c0 = r * F
                    if pairs == 4:
                        ph1a = ps1.tile([128, 1024], F32, tag="hp1")
                        ph1b = ps1.tile([128, 1024], F32, tag="hp1")
                        ph1 = [ph1a, ph1b]
                    else:
                        ph1t = ps1.tile([128, F * pairs], F32, tag="hp1")
                        ph1 = [ph1t]
                    S[i].update(ph1=ph1, c0=c0, pairs=pairs)
                    if stage != "evac":
                        for p in range(pairs):
                            nc.tensor.matmul(
                                ph1[p // 2][:, F * (p % 2) : F * (p % 2) + F],
                                w1_sb[32 * p : 32 * p + 6, :],
                                x_sb[32 * p : 32 * p + 6, c0 : c0 + F],
                                start=True,
                                stop=True,
                                tile_position=(32 * p, 0),
                            )

                def emit_evac1(i):  # DVE: bias+relu for both L1 halves
                    pairs = S[i]["pairs"]
                    ph1 = S[i]["ph1"]
                    h1r = h1pool.tile([128, F * pairs], BF16, tag="h1r")
                    S[i]["h1r"] = h1r
                    if stage == "pe":
                        return
                    if pairs == 4:
                        nc.vector.tensor_scalar(
                            h1r[:, 0:1024], ph1[0][:], b1_sb[:, 0:1],
                            0.0, ADD, MAX,
                        )
                        nc.vector.tensor_scalar(
                            h1r[:, 1024:2048], ph1[1][:], b1_sb[:, 0:1],
                            0.0, ADD, MAX,
                        )
                    else:
                        nc.vector.tensor_scalar(
                            h1r[:], ph1[0][:], b1_sb[:, 0:1], 0.0, ADD, MAX,
                        )

                def emit_L2(i):
                    pairs = S[i]["pairs"]
                    h1in = h1c if stage == "pe" else S[i]["h1r"]
                    if pairs == 4:
                        ph2a = ps2.tile([128, 1024], F32, tag="hp2")
                        ph2b = ps2.tile([128, 1024], F32, tag="hp2")
                        ph2 = [ph2a, ph2b]
                    else:
                        ph2t = ps2.tile([128, F * pairs], F32, tag="hp2")
                        ph2 = [ph2t]
                    S[i]["ph2"] = ph2
                    if stage != "evac":
                        for p in range(pairs):
                            nc.tensor.matmul(
                                ph2[p // 2][:, F * (p % 2) : F * (p % 2) + F],
                                w2_sb[:, :],
                                h1in[:, F * p : F * p + F],
                                start=True,
                                stop=True,
                                tile_position=(0, 0),
                            )

                def emit_evac2(i):  # ACT: bias+relu for both L2 halves
                    pairs = S[i]["pairs"]
                    ph2 = S[i]["ph2"]
                    h2r = h2pool.tile([128, F * pairs], BF16, tag="h2r")
                    S[i]["h2r"] = h2r
                    if stage == "pe":
                        return
                    if pairs == 4:
                        nc.scalar.activation(
                            h2r[:, 0:1024], ph2[0][:], RELU,
                            bias=b2_sb[:, 0:1],
                        )
                        nc.scalar.activation(
                            h2r[:, 1024:2048], ph2[1][:], RELU,
                            bias=b2_sb[:, 0:1],
                        )
                    else:
                        nc.vector.tensor_scalar(
                            h2r[:], ph2[0][:], b2_sb[:, 0:1], 0.0, ADD, MAX,
                        )

                def emit_L3(i):
                    pairs = S[i]["pairs"]
                    h2in = h2c if stage == "pe" else S[i]["h2r"]
                    yt = S[i]["ph2"][-1]
                    if stage != "evac":
                        for p in range(pairs):
                            nc.tensor.matmul(
                                yt[32 * p : 32 * p + 2, 0:F],
                                w3_sb[:, 0:2],
                                h2in[:, F * p : F * p + F],
                                start=True,
                                stop=True,
                                tile_position=(0, 32 * p),
                            )

                def emit_y(i):  # ACT: bias-add copy of y psum into y_sb
                    if stage == "pe":
                        return
                    yt = S[i]["ph2"][-1]
                    c0 = S[i]["c0"]
                    nc.scalar.activation(
                        y_sb[:, c0 : c0 + F], yt[:, 0:F],
                        IDENT, bias=b3_sb[:, 0:1]
                    )

                h = fr // 2  # first y-DMA wave covers rounds 0..h-1
                if stage == "pe":
                    h = 0  # y_sb never written; tiny debug output instead
                # software-pipelined emission: each engine's queue is ordered
                # so the head is (expected) always ready:
                #   PE : L1(0), L2(0), L1(1), L3(0), L2(1), L1(2), L3(1), ...
                #   DVE: L1a(i), L1b(i)
                #   ACT: y(i-1), L2a(i), L2b(i)
                emit_L1(0)
                for i in range(n):
                    emit_evac1(i)
                    if i >= 1:
                        emit_y(i - 1)
                    if i == h and h > 0:
                        for g in range(4):
                            nc.sync.dma_start(
                                yO.ap()[g][:, 0 : h * F],
                                y_sb[32 * g : 32 * g + 2, 0 : h * F],
                            )
                    emit_L2(i)
                    if i + 1 < n:
                        emit_L1(i + 1)
                    emit_evac2(i)
                    emit_L3(i)
                emit_y(n - 1)

                if stage == "pe":
                    nc.sync.dma_start(
                        yO.ap()[0][:, 0:16], h1c[0:2, 0:32].bitcast(F32)
                    )
                else:
                    for g in range(4):
                        nc.sync.dma_start(
                            yO.ap()[g][:, h * F :],
                            y_sb[32 * g : 32 * g + 2, h * F :],
                        )

    nc.compile()
    return nc


def kernel(x, extents_min, extents_max, W1, b1, W2, b2, W3, b3):
    global LAST_RESULTS, LAST_IN_MAPS, LAST_NC, LAST_CFG
    x = np.ascontiguousarray(np.asarray(x, dtype=np.float32))
    extents_min = np.asarray(extents_min, dtype=np.float32)
    extents_max = np.asarray(extents_max, dtype=np.float32)
    W1 = np.asarray(W1, dtype=np.float32)
    b1 = np.asarray(b1, dtype=np.float32)
    W2 = np.asarray(W2, dtype=np.float32)
    b2 = np.asarray(b2, dtype=np.float32)
    W3 = np.asarray(W3, dtype=np.float32)
    b3 = np.asarray(b3, dtype=np.float32)

    n_pts = x.shape[0]
    E = W1.shape[0]
    assert E == N_CORES

    # --- routing (identical fp32 math to the reference) ---
    gvec = np.asarray(GRID, dtype=np.float32)
    u = np.clip((x + np.float32(1.0)) * np.float32(0.5), 0.0, 0.99)
    gi = (u * gvec).astype(np.int32)
    idx = gi[:, 0] + gi[:, 1] * GRID[0] + gi[:, 2] * (GRID[0] * GRID[1])

    order = np.argsort(idx, kind="stable")
    counts = np.bincount(idx, minlength=E)
    starts = np.concatenate([[0], np.cumsum(counts)[:-1]])
    x_sorted = x[order]

    maxc = int(counts.max())
    fr = maxc // 4096
    tp = -(-(maxc - fr * 4096) // 1024)  # ceil
    if tp == 4:
        fr, tp = fr + 1, 0
    if fr == 0 and tp == 0:
        tp = 1
    cap = fr * 4096 + tp * 1024
    npairs = fr * 4 + tp
    nr = fr + (1 if tp else 0)

    # --- fold the expert-local normalization into layer-1 weights ---
    # xn = s*x + t, s = 2/(emax-emin), t = -2*emin/(emax-emin) - 1
    span = extents_max - extents_min          # [E, 3]
    s = 2.0 / span
    tvec = -2.0 * extents_min / span - 1.0
    W1p = W1 * s[:, :, None]                  # [E, 3, H]
    b1p = b1 + np.einsum("ec,ech->eh", tvec, W1)

    in_maps = []
    for e in range(E):
        xe = np.zeros((cap, 3), dtype=np.float32)
        xe[: counts[e]] = x_sorted[starts[e] : starts[e] + counts[e]]
        # xT[p, 3s+c, k*512+n] = xe[(4k+p)*1024 + s*512 + n, c]
        xq = xe.reshape(npairs, 2, F, 3)
        xt = np.zeros((4, 6, nr * F), dtype=ml_dtypes.bfloat16)
        for p in range(4):
            nq = fr + (1 if p < tp else 0)
            blk = xq[p::4]                     # [nq, 2, 512, 3]
            assert blk.shape[0] == nq
            # rows 3s+c, cols k*512+n
            xt[p, :, : nq * F] = (
                blk.transpose(1, 3, 0, 2).reshape(6, nq * F)
                .astype(ml_dtypes.bfloat16)
            )
        # w1: 4 row strips (one per pair), each the [6,128] block-diag of W1'
        w1e = W1p[e].astype(ml_dtypes.bfloat16)
        w1_full = np.zeros((128, 128), dtype=ml_dtypes.bfloat16)
        for p in range(4):
            w1_full[32 * p : 32 * p + 3, 0:64] = w1e
            w1_full[32 * p + 3 : 32 * p + 6, 64:128] = w1e
        # w2: [128,128] block-diag of W2
        w2_full = np.zeros((128, 128), dtype=ml_dtypes.bfloat16)
        w2_full[0:64, 0:64] = W2[e].astype(ml_dtypes.bfloat16)
        w2_full[64:128, 64:128] = W2[e].astype(ml_dtypes.bfloat16)
        # w3: [128,2] block-diag
        w3_full = np.zeros((128, 2), dtype=ml_dtypes.bfloat16)
        w3_full[0:64, 0] = W3[e, :, 0].astype(ml_dtypes.bfloat16)
        w3_full[64:128, 1] = W3[e, :, 0].astype(ml_dtypes.bfloat16)
        b1_full = np.tile(b1p[e], 2)[:, None].astype(np.float32)
        b2_full = np.tile(b2[e], 2)[:, None].astype(np.float32)
        b3_full = np.full((128, 1), b3[e, 0], dtype=np.float32)
        cst = np.concatenate(
            [
                w1_full.view(np.uint8),
                w2_full.view(np.uint8),
                w3_full.view(np.uint8),
                b1_full.view(np.uint8),
                b2_full.view(np.uint8),
                b3_full.view(np.uint8),
            ],
            axis=1,
        )
        assert cst.shape == (128, 528), cst.shape
        in_maps.append(
            {
                "xT": np.ascontiguousarray(xt),
                "cst": np.ascontiguousarray(cst),
            }
        )

    cfg = (fr, tp)
    if cfg not in _PROGRAM_CACHE:
        _PROGRAM_CACHE[cfg] = _build_program(cfg)
    nc = _PROGRAM_CACHE[cfg]

    res = run_bass_kernel_spmd(nc, in_maps, core_ids=list(range(N_CORES)))
    LAST_RESULTS = res
    LAST_IN_MAPS = in_maps
    LAST_NC = nc
    LAST_CFG = cfg

    # --- unshard: y_dev[p, s, k*512+n] -> point (4k+p)*1024 + s*512 + n ---
    y_sorted = np.empty(n_pts, dtype=np.float32)
    for e in range(E):
        ydev = res.results[e]["y"]             # [4, 2, nr*512]
        ye = np.empty((npairs, 1024), dtype=np.float32)
        for p in range(4):
            nq = fr + (1 if p < tp else 0)
            blk = ydev[p, :, : nq * F].reshape(2, nq, F)
            ye[p::4] = blk.transpose(1, 0, 2).reshape(nq, 1024)
        yflat = ye.reshape(cap)
        y_sorted[starts[e] : starts[e] + counts[e]] = yflat[: counts[e]]

    y_full = np.empty(n_pts, dtype=np.float32)
    y_full[order] = y_sorted
    return y_full[:, None]


# revision 22
# speedup vs baseline: 1.3712x; 1.0245x over previous
"""Ensemble-SRN MoE routing kernel for 8 TRN2 NeuronCores.

Strategy: expert-parallel sharding. The 8 experts are axis-aligned octants of
[-1,1]^3 (GRID=(2,2,2)); core e receives exactly the points routed to expert e
(the all-to-all dispatch happens on the host as part of sharding), runs a dense
single-expert 3->64->64->1 ReLU MLP over its (padded) shard, and the host
inverse-permutes the outputs.

Device kernel (per core, fr rounds of 4096 points = 4 pairs x 1024, plus a
tail of tp pairs sized from the actual max shard count):
  - All x coordinates are preloaded into SBUF once (4 DMAs, one per 6-row
    partition strip) and all y stays SBUF-resident until 2 waves of 4 output
    DMAs; per-round DMA traffic is zero.  (HWDGE costs ~625ns per dma_start,
    so per-round DMAs were the original top bottleneck.)
  - All weights/biases are packed into one 528-byte-per-partition uint8
    tensor, DMA'd once, and accessed through bitcast views.
  - L1 (3->64): 4 strip matmuls [6,128]x[6,512] at tile_position (32p, 0),
    two point-tiles block-diagonal per matmul.
  - L2 (64->64): 4 full-array matmuls with W2 block-diagonal twice.
  - L3 (64->1): 4 matmuls with M=2 block-diag w3 ([128,2]) at tile_position
    (0,32p); y for pair p lands at PSUM partitions {32p,32p+1} of bank 0 of
    ph2b (reused after its evacuation).
  - Evacuation split: DVE (tensor_scalar add-bias+relu) takes both L1
    halves; ACT (activation relu/identity with bias) takes both L2 halves
    plus the y evac.  Per round: DVE 2x[128,1024], ACT 2x[128,1024] +
    [128,512] - the ACT chain (~2.7us) is the steady-state critical path.
  - PSUM: ph1 pool 2x[128,1024]f32 + ph2 pool 2x[128,1024]f32 = all 8 banks;
    emission is software-pipelined (y evac of round i-1 ordered before round
    i's L2 evacs) so ACT streams without waiting on L3.
  - CAUTION: multi-partition-dim DMA APs (rearrange "(g sub) f -> g sub f")
    are silently wrong on hardware; only plain contiguous-partition slices
    are used for DMA.
"""

import ml_dtypes
import numpy as np

import concourse.bass as bass
import concourse.tile as tile
from concourse import bacc, mybir
from concourse.bass_utils import run_bass_kernel_spmd

F32 = mybir.dt.float32
BF16 = mybir.dt.bfloat16
U8 = mybir.dt.uint8

N_CORES = 8
GRID = (2, 2, 2)
H = 64
F = 512              # points per tile (one PSUM-bank free dim, fp32)

_PROGRAM_CACHE = {}
LAST_RESULTS = None  # BassKernelResults of the last run (for test harness)
LAST_IN_MAPS = None  # per-core input dicts of the last run (for test harness)
LAST_NC = None       # compiled program of the last run (for test harness)
LAST_CFG = None      # (fr, tp) of the last run (for test harness)


def _build_program(cfg, loop_n=None, stage="full"):
    """Build the SPMD program for cfg=(full_rounds, tail_pairs). loop_n
    (bench only): repeat the whole body loop_n times in a hardware For_i so
    device time can be measured through the noisy axon dispatch path by
    differencing two loop counts."""
    fr, tp = cfg
    nr = fr + (1 if tp else 0)          # column blocks in x/y SBUF tiles
    nc = bacc.Bacc(
        "TRN2",
        target_bir_lowering=False,
        debug=False,
        num_devices=N_CORES,
    )
    xT = nc.dram_tensor("xT", [4, 6, nr * F], BF16, kind="ExternalInput")
    # packed constants, one DMA: w1[0:256) w2[256:512) w3[512:516) b1[516:520)
    # b2[520:524) b3[524:528) bytes per partition
    cst = nc.dram_tensor("cst", [128, 528], U8, kind="ExternalInput")
    yO = nc.dram_tensor("y", [4, 2, nr * F], F32, kind="ExternalOutput")

    RELU = mybir.ActivationFunctionType.Relu
    IDENT = mybir.ActivationFunctionType.Identity
    ADD = mybir.AluOpType.add
    MAX = mybir.AluOpType.max

    with tile.TileContext(nc) as tc:
        with (
            tc.tile_pool(name="const", bufs=1) as const,
            tc.tile_pool(name="xall", bufs=1) as xall,
            tc.tile_pool(name="yall", bufs=1) as yall,
            tc.tile_pool(name="h1p", bufs=2) as h1pool,
            tc.tile_pool(name="h2p", bufs=2) as h2pool,
            tc.tile_pool(name="ps1", bufs=2, space="PSUM") as ps1,
            tc.tile_pool(name="ps2", bufs=2, space="PSUM") as ps2,
        ):
            c_sb = const.tile([128, 528], U8)
            nc.sync.dma_start(c_sb[:], cst.ap())
            w1_sb = c_sb[:, 0:256].bitcast(BF16)
            w2_sb = c_sb[:, 256:512].bitcast(BF16)
            w3_sb = c_sb[:, 512:516].bitcast(BF16)
            b1_sb = c_sb[:, 516:520].bitcast(F32)
            b2_sb = c_sb[:, 520:524].bitcast(F32)
            b3_sb = c_sb[:, 524:528].bitcast(F32)

            import contextlib
            loop_cm = (
                tc.For_i(
                    0, loop_n, 1,
                    hint_engines=(
                        mybir.EngineType.PE,
                        mybir.EngineType.DVE,
                        mybir.EngineType.Activation,
                        mybir.EngineType.SP,
                    ),
                )
                if loop_n
                else contextlib.nullcontext()
            )
            with loop_cm:
                x_sb = xall.tile([128, nr * F], BF16)
                # NOTE: multi-partition-dim APs (rearrange "(g sub) f") are
                # silently wrong through the DMA path on HW - use 4 plain
                # contiguous-partition DMAs.
                for p in range(4):
                    eng = nc.sync if p % 2 == 0 else nc.scalar
                    eng.dma_start(
                        x_sb[32 * p : 32 * p + 6, :], xT.ap()[p]
                    )
                y_sb = yall.tile([128, nr * F], F32)

                # constant h tiles for stage="pe" (decouple MMs from evacs)
                if stage == "pe":
                    h1c = const.tile([128, 2048], BF16)
                    nc.vector.memset(h1c[:], 1.0)
                    h2c = const.tile([128, 2048], BF16)
                    nc.vector.memset(h2c[:], 1.0)

                rounds = [(r, 4) for r in range(fr)]
                if tp:
                    rounds.append((fr, tp))
                n = len(rounds)
                S = [dict() for _ in range(n)]

                def emit_L1(i):
                    r, pairs = rounds[i]
                    c0 = r * F
                    if pairs == 4:
                        ph1a = ps1.tile([128, 1024], F32, tag="hp1")
                        ph1b = ps1.tile([128, 1024], F32, tag="hp1")
                        ph1 = [ph1a, ph1b]
                    else:
                        ph1t = ps1.tile([128, F * pairs], F32, tag="hp1")
                        ph1 = [ph1t]
                    S[i].update(ph1=ph1, c0=c0, pairs=pairs)
                    if stage != "evac":
                        for p in range(pairs):
                            nc.tensor.matmul(
                                ph1[p // 2][:, F * (p % 2) : F * (p % 2) + F],
                                w1_sb[32 * p : 32 * p + 6, :],
                                x_sb[32 * p : 32 * p + 6, c0 : c0 + F],
                                start=True,
                                stop=True,
                                tile_position=(32 * p, 0),
                            )

                def emit_evac1(i):  # DVE: bias+relu for both L1 halves
                    pairs = S[i]["pairs"]
                    ph1 = S[i]["ph1"]
                    h1r = h1pool.tile([128, F * pairs], BF16, tag="h1r")
                    S[i]["h1r"] = h1r
                    if stage == "pe":
                        return
                    if pairs == 4:
                        nc.vector.tensor_scalar(
                            h1r[:, 0:1024], ph1[0][:], b1_sb[:, 0:1],
                            0.0, ADD, MAX,
                        )
                        nc.vector.tensor_scalar(
                            h1r[:, 1024:2048], ph1[1][:], b1_sb[:, 0:1],
                            0.0, ADD, MAX,
                        )
                    else:
                        nc.vector.tensor_scalar(
                            h1r[:], ph1[0][:], b1_sb[:, 0:1], 0.0, ADD, MAX,
                        )

                def emit_L2(i):
                    pairs = S[i]["pairs"]
                    h1in = h1c if stage == "pe" else S[i]["h1r"]
                    if pairs == 4:
                        ph2a = ps2.tile([128, 1024], F32, tag="hp2")
                        ph2b = ps2.tile([128, 1024], F32, tag="hp2")
                        ph2 = [ph2a, ph2b]
                    else:
                        ph2t = ps2.tile([128, F * pairs], F32, tag="hp2")
                        ph2 = [ph2t]
                    S[i]["ph2"] = ph2
                    if stage != "evac":
                        for p in range(pairs):
                            nc.tensor.matmul(
                                ph2[p // 2][:, F * (p % 2) : F * (p % 2) + F],
                                w2_sb[:, :],
                                h1in[:, F * p : F * p + F],
                                start=True,
                                stop=True,
                                tile_position=(0, 0),
                            )

                def emit_evac2(i):  # ACT: bias+relu for both L2 halves
                    pairs = S[i]["pairs"]
                    ph2 = S[i]["ph2"]
                    h2r = h2pool.tile([128, F * pairs], BF16, tag="h2r")
                    S[i]["h2r"] = h2r
                    if stage == "pe":
                        return
                    if pairs == 4:
                        nc.scalar.activation(
                            h2r[:, 0:1024], ph2[0][:], RELU,
                            bias=b2_sb[:, 0:1],
                        )
                        nc.scalar.activation(
                            h2r[:, 1024:2048], ph2[1][:], RELU,
                            bias=b2_sb[:, 0:1],
                        )
                    else:
                        nc.vector.tensor_scalar(
                            h2r[:], ph2[0][:], b2_sb[:, 0:1], 0.0, ADD, MAX,
                        )

                def emit_L3(i):
                    pairs = S[i]["pairs"]
                    h2in = h2c if stage == "pe" else S[i]["h2r"]
                    yt = S[i]["ph2"][-1]
                    if stage != "evac":
                        for p in range(pairs):
                            nc.tensor.matmul(
                                yt[32 * p : 32 * p + 2, 0:F],
                                w3_sb[:, 0:2],
                                h2in[:, F * p : F * p + F],
                                start=True,
                                stop=True,
                                tile_position=(0, 32 * p),
                            )

                def emit_y(i):  # ACT: bias-add copy of y psum into y_sb
                    if stage == "pe":
                        return
                    yt = S[i]["ph2"][-1]
                    c0 = S[i]["c0"]
                    nc.scalar.activation(
                        y_sb[:, c0 : c0 + F], yt[:, 0:F],
                        IDENT, bias=b3_sb[:, 0:1]
                    )

                h = fr // 2  # first y-DMA wave covers rounds 0..h-1
                if stage == "pe":
                    h = 0  # y_sb never written; tiny debug output instead
                # software-pipelined emission: each engine's queue is ordered
                # so the head is (expected) always ready:
                #   PE : L1(0), L2(0), L1(1), L3(0), L2(1), L1(2), L3(1), ...
                #   DVE: L1a(i), L1b(i)
                #   ACT: y(i-1), L2a(i), L2b(i)
                emit_L1(0)
                for i in range(n):
                    emit_evac1(i)
                    if i >= 1:
                        emit_y(i - 1)
                    if i == h and h > 0:
                        for g in range(4):
                            nc.sync.dma_start(
                                yO.ap()[g][:, 0 : h * F],
                                y_sb[32 * g : 32 * g + 2, 0 : h * F],
                            )
                    emit_L2(i)
                    if i + 1 < n:
                        emit_L1(i + 1)
                    emit_evac2(i)
                    emit_L3(i)
                emit_y(n - 1)

                if stage == "pe":
                    nc.sync.dma_start(
                        yO.ap()[0][:, 0:16], h1c[0:2, 0:32].bitcast(F32)
                    )
                else:
                    for g in range(4):
                        eng = nc.sync if g % 2 == 0 else nc.scalar
                        eng.dma_start(
                            yO.ap()[g][:, h * F :],
                            y_sb[32 * g : 32 * g + 2, h * F :],
                        )

    nc.compile()
    return nc


def kernel(x, extents_min, extents_max, W1, b1, W2, b2, W3, b3):
    global LAST_RESULTS, LAST_IN_MAPS, LAST_NC, LAST_CFG
    x = np.ascontiguousarray(np.asarray(x, dtype=np.float32))
    extents_min = np.asarray(extents_min, dtype=np.float32)
    extents_max = np.asarray(extents_max, dtype=np.float32)
    W1 = np.asarray(W1, dtype=np.float32)
    b1 = np.asarray(b1, dtype=np.float32)
    W2 = np.asarray(W2, dtype=np.float32)
    b2 = np.asarray(b2, dtype=np.float32)
    W3 = np.asarray(W3, dtype=np.float32)
    b3 = np.asarray(b3, dtype=np.float32)

    n_pts = x.shape[0]
    E = W1.shape[0]
    assert E == N_CORES

    # --- routing (identical fp32 math to the reference) ---
    gvec = np.asarray(GRID, dtype=np.float32)
    u = np.clip((x + np.float32(1.0)) * np.float32(0.5), 0.0, 0.99)
    gi = (u * gvec).astype(np.int32)
    idx = gi[:, 0] + gi[:, 1] * GRID[0] + gi[:, 2] * (GRID[0] * GRID[1])

    order = np.argsort(idx, kind="stable")
    counts = np.bincount(idx, minlength=E)
    starts = np.concatenate([[0], np.cumsum(counts)[:-1]])
    x_sorted = x[order]

    maxc = int(counts.max())
    fr = maxc // 4096
    tp = -(-(maxc - fr * 4096) // 1024)  # ceil
    if tp == 4:
        fr, tp = fr + 1, 0
    if fr == 0 and tp == 0:
        tp = 1
    cap = fr * 4096 + tp * 1024
    npairs = fr * 4 + tp
    nr = fr + (1 if tp else 0)

    # --- fold the expert-local normalization into layer-1 weights ---
    # xn = s*x + t, s = 2/(emax-emin), t = -2*emin/(emax-emin) - 1
    span = extents_max - extents_min          # [E, 3]
    s = 2.0 / span
    tvec = -2.0 * extents_min / span - 1.0
    W1p = W1 * s[:, :, None]                  # [E, 3, H]
    b1p = b1 + np.einsum("ec,ech->eh", tvec, W1)

    in_maps = []
    for e in range(E):
        xe = np.zeros((cap, 3), dtype=np.float32)
        xe[: counts[e]] = x_sorted[starts[e] : starts[e] + counts[e]]
        # xT[p, 3s+c, k*512+n] = xe[(4k+p)*1024 + s*512 + n, c]
        xq = xe.reshape(npairs, 2, F, 3)
        xt = np.zeros((4, 6, nr * F), dtype=ml_dtypes.bfloat16)
        for p in range(4):
            nq = fr + (1 if p < tp else 0)
            blk = xq[p::4]                     # [nq, 2, 512, 3]
            assert blk.shape[0] == nq
            # rows 3s+c, cols k*512+n
            xt[p, :, : nq * F] = (
                blk.transpose(1, 3, 0, 2).reshape(6, nq * F)
                .astype(ml_dtypes.bfloat16)
            )
        # w1: 4 row strips (one per pair), each the [6,128] block-diag of W1'
        w1e = W1p[e].astype(ml_dtypes.bfloat16)
        w1_full = np.zeros((128, 128), dtype=ml_dtypes.bfloat16)
        for p in range(4):
            w1_full[32 * p : 32 * p + 3, 0:64] = w1e
            w1_full[32 * p + 3 : 32 * p + 6, 64:128] = w1e
        # w2: [128,128] block-diag of W2
        w2_full = np.zeros((128, 128), dtype=ml_dtypes.bfloat16)
        w2_full[0:64, 0:64] = W2[e].astype(ml_dtypes.bfloat16)
        w2_full[64:128, 64:128] = W2[e].astype(ml_dtypes.bfloat16)
        # w3: [128,2] block-diag
        w3_full = np.zeros((128, 2), dtype=ml_dtypes.bfloat16)
        w3_full[0:64, 0] = W3[e, :, 0].astype(ml_dtypes.bfloat16)
        w3_full[64:128, 1] = W3[e, :, 0].astype(ml_dtypes.bfloat16)
        b1_full = np.tile(b1p[e], 2)[:, None].astype(np.float32)
        b2_full = np.tile(b2[e], 2)[:, None].astype(np.float32)
        b3_full = np.full((128, 1), b3[e, 0], dtype=np.float32)
        cst = np.concatenate(
            [
                w1_full.view(np.uint8),
                w2_full.view(np.uint8),
                w3_full.view(np.uint8),
                b1_full.view(np.uint8),
                b2_full.view(np.uint8),
                b3_full.view(np.uint8),
            ],
            axis=1,
        )
        assert cst.shape == (128, 528), cst.shape
        in_maps.append(
            {
                "xT": np.ascontiguousarray(xt),
                "cst": np.ascontiguousarray(cst),
            }
        )

    cfg = (fr, tp)
    if cfg not in _PROGRAM_CACHE:
        _PROGRAM_CACHE[cfg] = _build_program(cfg)
    nc = _PROGRAM_CACHE[cfg]

    res = run_bass_kernel_spmd(nc, in_maps, core_ids=list(range(N_CORES)))
    LAST_RESULTS = res
    LAST_IN_MAPS = in_maps
    LAST_NC = nc
    LAST_CFG = cfg

    # --- unshard: y_dev[p, s, k*512+n] -> point (4k+p)*1024 + s*512 + n ---
    y_sorted = np.empty(n_pts, dtype=np.float32)
    for e in range(E):
        ydev = res.results[e]["y"]             # [4, 2, nr*512]
        ye = np.empty((npairs, 1024), dtype=np.float32)
        for p in range(4):
            nq = fr + (1 if p < tp else 0)
            blk = ydev[p, :, : nq * F].reshape(2, nq, F)
            ye[p::4] = blk.transpose(1, 0, 2).reshape(nq, 1024)
        yflat = ye.reshape(cap)
        y_sorted[starts[e] : starts[e] + counts[e]] = yflat[: counts[e]]

    y_full = np.empty(n_pts, dtype=np.float32)
    y_full[order] = y_sorted
    return y_full[:, None]


# revision 23
# speedup vs baseline: 1.4412x; 1.0511x over previous
"""Ensemble-SRN MoE routing kernel for 8 TRN2 NeuronCores.

Strategy: expert-parallel sharding. The 8 experts are axis-aligned octants of
[-1,1]^3 (GRID=(2,2,2)); core e receives exactly the points routed to expert e
(the all-to-all dispatch happens on the host as part of sharding), runs a dense
single-expert 3->64->64->1 ReLU MLP over its (padded) shard, and the host
inverse-permutes the outputs.

Device kernel (per core, fr rounds of 4096 points = 4 pairs x 1024, plus a
tail of tp pairs sized from the actual max shard count):
  - All x coordinates are preloaded into SBUF once (4 DMAs, one per 6-row
    partition strip) and all y stays SBUF-resident until 2 waves of 4 output
    DMAs; per-round DMA traffic is zero.  (HWDGE costs ~625ns per dma_start,
    so per-round DMAs were the original top bottleneck.)
  - All weights/biases are packed into one 528-byte-per-partition uint8
    tensor, DMA'd once, and accessed through bitcast views.
  - L1 (3->64): 4 strip matmuls [6,128]x[6,512] at tile_position (32p, 0),
    two point-tiles block-diagonal per matmul.
  - L2 (64->64): 4 full-array matmuls with W2 block-diagonal twice.
  - L3 (64->1): 4 matmuls with M=2 block-diag w3 ([128,2]) at tile_position
    (0,32p); y for pair p lands at PSUM partitions {32p,32p+1} of bank 0 of
    ph2b (reused after its evacuation).
  - Evacuation split: DVE (tensor_scalar add-bias+relu) takes both L1
    halves; ACT (activation relu/identity with bias) takes both L2 halves
    plus the y evac.  Per round: DVE 2x[128,1024], ACT 2x[128,1024] +
    [128,512] - the ACT chain (~2.7us) is the steady-state critical path.
  - PSUM: ph1 pool 2x[128,1024]f32 + ph2 pool 2x[128,1024]f32 = all 8 banks;
    emission is software-pipelined (y evac of round i-1 ordered before round
    i's L2 evacs) so ACT streams without waiting on L3.
  - CAUTION: multi-partition-dim DMA APs (rearrange "(g sub) f -> g sub f")
    are silently wrong on hardware; only plain contiguous-partition slices
    are used for DMA.
"""

import ml_dtypes
import numpy as np

import concourse.bass as bass
import concourse.tile as tile
from concourse import bacc, mybir
from concourse.bass_utils import run_bass_kernel_spmd

F32 = mybir.dt.float32
BF16 = mybir.dt.bfloat16
U8 = mybir.dt.uint8

N_CORES = 8
GRID = (2, 2, 2)
H = 64
F = 512              # points per tile (one PSUM-bank free dim, fp32)

_PROGRAM_CACHE = {}
LAST_RESULTS = None  # BassKernelResults of the last run (for test harness)
LAST_IN_MAPS = None  # per-core input dicts of the last run (for test harness)
LAST_NC = None       # compiled program of the last run (for test harness)
LAST_CFG = None      # (fr, tp) of the last run (for test harness)


def _build_program(cfg, loop_n=None, stage="full"):
    """Build the SPMD program for cfg=(full_rounds, tail_pairs). loop_n
    (bench only): repeat the whole body loop_n times in a hardware For_i so
    device time can be measured through the noisy axon dispatch path by
    differencing two loop counts."""
    fr, tp = cfg
    nr = fr + (1 if tp else 0)          # column blocks in x/y SBUF tiles
    nc = bacc.Bacc(
        "TRN2",
        target_bir_lowering=False,
        debug=False,
        num_devices=N_CORES,
    )
    xT = nc.dram_tensor("xT", [4, 6, nr * F], BF16, kind="ExternalInput")
    # packed constants, one DMA: w1[0:256) w2[256:512) w3[512:516) b1[516:520)
    # b2[520:524) b3[524:528) bytes per partition
    cst = nc.dram_tensor("cst", [128, 528], U8, kind="ExternalInput")
    yO = nc.dram_tensor("y", [4, 2, nr * F], F32, kind="ExternalOutput")

    RELU = mybir.ActivationFunctionType.Relu
    IDENT = mybir.ActivationFunctionType.Identity
    ADD = mybir.AluOpType.add
    MAX = mybir.AluOpType.max

    with tile.TileContext(nc) as tc:
        with (
            tc.tile_pool(name="const", bufs=1) as const,
            tc.tile_pool(name="xall", bufs=1) as xall,
            tc.tile_pool(name="yall", bufs=1) as yall,
            tc.tile_pool(name="h1p", bufs=3) as h1pool,
            tc.tile_pool(name="h2p", bufs=3) as h2pool,
            tc.tile_pool(name="ps1", bufs=2, space="PSUM") as ps1,
            tc.tile_pool(name="ps2", bufs=2, space="PSUM") as ps2,
        ):
            c_sb = const.tile([128, 528], U8)
            nc.sync.dma_start(c_sb[:], cst.ap())
            w1_sb = c_sb[:, 0:256].bitcast(BF16)
            w2_sb = c_sb[:, 256:512].bitcast(BF16)
            w3_sb = c_sb[:, 512:516].bitcast(BF16)
            b1_sb = c_sb[:, 516:520].bitcast(F32)
            b2_sb = c_sb[:, 520:524].bitcast(F32)
            b3_sb = c_sb[:, 524:528].bitcast(F32)

            import contextlib
            loop_cm = (
                tc.For_i(
                    0, loop_n, 1,
                    hint_engines=(
                        mybir.EngineType.PE,
                        mybir.EngineType.DVE,
                        mybir.EngineType.Activation,
                        mybir.EngineType.SP,
                    ),
                )
                if loop_n
                else contextlib.nullcontext()
            )
            with loop_cm:
                x_sb = xall.tile([128, nr * F], BF16)
                # NOTE: multi-partition-dim APs (rearrange "(g sub) f") are
                # silently wrong through the DMA path on HW - use 4 plain
                # contiguous-partition DMAs.
                for p in range(4):
                    eng = nc.scalar if p % 2 == 0 else nc.sync
                    eng.dma_start(
                        x_sb[32 * p : 32 * p + 6, :], xT.ap()[p]
                    )
                y_sb = yall.tile([128, nr * F], F32)

                # constant h tiles for stage="pe" (decouple MMs from evacs)
                if stage == "pe":
                    h1c = const.tile([128, 2048], BF16)
                    nc.vector.memset(h1c[:], 1.0)
                    h2c = const.tile([128, 2048], BF16)
                    nc.vector.memset(h2c[:], 1.0)

                rounds = [(r, 4) for r in range(fr)]
                if tp:
                    rounds.append((fr, tp))
                n = len(rounds)
                S = [dict() for _ in range(n)]

                def emit_L1(i):
                    r, pairs = rounds[i]
                    c0 = r * F
                    if pairs == 4:
                        ph1a = ps1.tile([128, 1024], F32, tag="hp1")
                        ph1b = ps1.tile([128, 1024], F32, tag="hp1")
                        ph1 = [ph1a, ph1b]
                    else:
                        ph1t = ps1.tile([128, F * pairs], F32, tag="hp1")
                        ph1 = [ph1t]
                    S[i].update(ph1=ph1, c0=c0, pairs=pairs)
                    if stage != "evac":
                        for p in range(pairs):
                            nc.tensor.matmul(
                                ph1[p // 2][:, F * (p % 2) : F * (p % 2) + F],
                                w1_sb[32 * p : 32 * p + 6, :],
                                x_sb[32 * p : 32 * p + 6, c0 : c0 + F],
                                start=True,
                                stop=True,
                                tile_position=(32 * p, 0),
                            )

                def emit_evac1(i):  # DVE: bias+relu for both L1 halves
                    pairs = S[i]["pairs"]
                    ph1 = S[i]["ph1"]
                    h1r = h1pool.tile([128, F * pairs], BF16, tag="h1r")
                    S[i]["h1r"] = h1r
                    if stage == "pe":
                        return
                    if pairs == 4:
                        nc.vector.tensor_scalar(
                            h1r[:, 0:1024], ph1[0][:], b1_sb[:, 0:1],
                            0.0, ADD, MAX,
                        )
                        nc.vector.tensor_scalar(
                            h1r[:, 1024:2048], ph1[1][:], b1_sb[:, 0:1],
                            0.0, ADD, MAX,
                        )
                    else:
                        nc.vector.tensor_scalar(
                            h1r[:], ph1[0][:], b1_sb[:, 0:1], 0.0, ADD, MAX,
                        )

                def emit_L2(i):
                    pairs = S[i]["pairs"]
                    h1in = h1c if stage == "pe" else S[i]["h1r"]
                    if pairs == 4:
                        ph2a = ps2.tile([128, 1024], F32, tag="hp2")
                        ph2b = ps2.tile([128, 1024], F32, tag="hp2")
                        ph2 = [ph2a, ph2b]
                    else:
                        ph2t = ps2.tile([128, F * pairs], F32, tag="hp2")
                        ph2 = [ph2t]
                    S[i]["ph2"] = ph2
                    if stage != "evac":
                        for p in range(pairs):
                            nc.tensor.matmul(
                                ph2[p // 2][:, F * (p % 2) : F * (p % 2) + F],
                                w2_sb[:, :],
                                h1in[:, F * p : F * p + F],
                                start=True,
                                stop=True,
                                tile_position=(0, 0),
                            )

                def emit_evac2(i):  # ACT: bias+relu for both L2 halves
                    pairs = S[i]["pairs"]
                    ph2 = S[i]["ph2"]
                    h2r = h2pool.tile([128, F * pairs], BF16, tag="h2r")
                    S[i]["h2r"] = h2r
                    if stage == "pe":
                        return
                    if pairs == 4:
                        nc.scalar.activation(
                            h2r[:, 0:1024], ph2[0][:], RELU,
                            bias=b2_sb[:, 0:1],
                        )
                        nc.scalar.activation(
                            h2r[:, 1024:2048], ph2[1][:], RELU,
                            bias=b2_sb[:, 0:1],
                        )
                    else:
                        nc.vector.tensor_scalar(
                            h2r[:], ph2[0][:], b2_sb[:, 0:1], 0.0, ADD, MAX,
                        )

                def emit_L3(i):
                    pairs = S[i]["pairs"]
                    h2in = h2c if stage == "pe" else S[i]["h2r"]
                    yt = S[i]["ph2"][-1]
                    if stage != "evac":
                        for p in range(pairs):
                            nc.tensor.matmul(
                                yt[32 * p : 32 * p + 2, 0:F],
                                w3_sb[:, 0:2],
                                h2in[:, F * p : F * p + F],
                                start=True,
                                stop=True,
                                tile_position=(0, 32 * p),
                            )

                def emit_y(i):  # ACT: bias-add copy of y psum into y_sb
                    if stage == "pe":
                        return
                    yt = S[i]["ph2"][-1]
                    c0 = S[i]["c0"]
                    nc.scalar.activation(
                        y_sb[:, c0 : c0 + F], yt[:, 0:F],
                        IDENT, bias=b3_sb[:, 0:1]
                    )

                h = fr // 2  # first y-DMA wave covers rounds 0..h-1
                if stage == "pe":
                    h = 0  # y_sb never written; tiny debug output instead
                # software-pipelined emission: each engine's queue is ordered
                # so the head is (expected) always ready:
                #   PE : L1(0), L2(0), L1(1), L3(0), L2(1), L1(2), L3(1), ...
                #   DVE: L1a(i), L1b(i)
                #   ACT: y(i-1), L2a(i), L2b(i)
                emit_L1(0)
                for i in range(n):
                    emit_evac1(i)
                    if i >= 1:
                        emit_y(i - 1)
                    if i == h and h > 0:
                        for g in range(4):
                            nc.sync.dma_start(
                                yO.ap()[g][:, 0 : h * F],
                                y_sb[32 * g : 32 * g + 2, 0 : h * F],
                            )
                    emit_L2(i)
                    if i + 1 < n:
                        emit_L1(i + 1)
                    emit_evac2(i)
                    emit_L3(i)
                emit_y(n - 1)

                if stage == "pe":
                    nc.sync.dma_start(
                        yO.ap()[0][:, 0:16], h1c[0:2, 0:32].bitcast(F32)
                    )
                else:
                    for g in range(4):
                        eng = nc.sync if g % 2 == 0 else nc.scalar
                        eng.dma_start(
                            yO.ap()[g][:, h * F :],
                            y_sb[32 * g : 32 * g + 2, h * F :],
                        )

    nc.compile()
    return nc


def kernel(x, extents_min, extents_max, W1, b1, W2, b2, W3, b3):
    global LAST_RESULTS, LAST_IN_MAPS, LAST_NC, LAST_CFG
    x = np.ascontiguousarray(np.asarray(x, dtype=np.float32))
    extents_min = np.asarray(extents_min, dtype=np.float32)
    extents_max = np.asarray(extents_max, dtype=np.float32)
    W1 = np.asarray(W1, dtype=np.float32)
    b1 = np.asarray(b1, dtype=np.float32)
    W2 = np.asarray(W2, dtype=np.float32)
    b2 = np.asarray(b2, dtype=np.float32)
    W3 = np.asarray(W3, dtype=np.float32)
    b3 = np.asarray(b3, dtype=np.float32)

    n_pts = x.shape[0]
    E = W1.shape[0]
    assert E == N_CORES

    # --- routing (identical fp32 math to the reference) ---
    gvec = np.asarray(GRID, dtype=np.float32)
    u = np.clip((x + np.float32(1.0)) * np.float32(0.5), 0.0, 0.99)
    gi = (u * gvec).astype(np.int32)
    idx = gi[:, 0] + gi[:, 1] * GRID[0] + gi[:, 2] * (GRID[0] * GRID[1])

    order = np.argsort(idx, kind="stable")
    counts = np.bincount(idx, minlength=E)
    starts = np.concatenate([[0], np.cumsum(counts)[:-1]])
    x_sorted = x[order]

    maxc = int(counts.max())
    fr = maxc // 4096
    tp = -(-(maxc - fr * 4096) // 1024)  # ceil
    if tp == 4:
        fr, tp = fr + 1, 0
    if fr == 0 and tp == 0:
        tp = 1
    cap = fr * 4096 + tp * 1024
    npairs = fr * 4 + tp
    nr = fr + (1 if tp else 0)

    # --- fold the expert-local normalization into layer-1 weights ---
    # xn = s*x + t, s = 2/(emax-emin), t = -2*emin/(emax-emin) - 1
    span = extents_max - extents_min          # [E, 3]
    s = 2.0 / span
    tvec = -2.0 * extents_min / span - 1.0
    W1p = W1 * s[:, :, None]                  # [E, 3, H]
    b1p = b1 + np.einsum("ec,ech->eh", tvec, W1)

    in_maps = []
    for e in range(E):
        xe = np.zeros((cap, 3), dtype=np.float32)
        xe[: counts[e]] = x_sorted[starts[e] : starts[e] + counts[e]]
        # xT[p, 3s+c, k*512+n] = xe[(4k+p)*1024 + s*512 + n, c]
        xq = xe.reshape(npairs, 2, F, 3)
        xt = np.zeros((4, 6, nr * F), dtype=ml_dtypes.bfloat16)
        for p in range(4):
            nq = fr + (1 if p < tp else 0)
            blk = xq[p::4]                     # [nq, 2, 512, 3]
            assert blk.shape[0] == nq
            # rows 3s+c, cols k*512+n
            xt[p, :, : nq * F] = (
                blk.transpose(1, 3, 0, 2).reshape(6, nq * F)
                .astype(ml_dtypes.bfloat16)
            )
        # w1: 4 row strips (one per pair), each the [6,128] block-diag of W1'
        w1e = W1p[e].astype(ml_dtypes.bfloat16)
        w1_full = np.zeros((128, 128), dtype=ml_dtypes.bfloat16)
        for p in range(4):
            w1_full[32 * p : 32 * p + 3, 0:64] = w1e
            w1_full[32 * p + 3 : 32 * p + 6, 64:128] = w1e
        # w2: [128,128] block-diag of W2
        w2_full = np.zeros((128, 128), dtype=ml_dtypes.bfloat16)
        w2_full[0:64, 0:64] = W2[e].astype(ml_dtypes.bfloat16)
        w2_full[64:128, 64:128] = W2[e].astype(ml_dtypes.bfloat16)
        # w3: [128,2] block-diag
        w3_full = np.zeros((128, 2), dtype=ml_dtypes.bfloat16)
        w3_full[0:64, 0] = W3[e, :, 0].astype(ml_dtypes.bfloat16)
        w3_full[64:128, 1] = W3[e, :, 0].astype(ml_dtypes.bfloat16)
        b1_full = np.tile(b1p[e], 2)[:, None].astype(np.float32)
        b2_full = np.tile(b2[e], 2)[:, None].astype(np.float32)
        b3_full = np.full((128, 1), b3[e, 0], dtype=np.float32)
        cst = np.concatenate(
            [
                w1_full.view(np.uint8),
                w2_full.view(np.uint8),
                w3_full.view(np.uint8),
                b1_full.view(np.uint8),
                b2_full.view(np.uint8),
                b3_full.view(np.uint8),
            ],
            axis=1,
        )
        assert cst.shape == (128, 528), cst.shape
        in_maps.append(
            {
                "xT": np.ascontiguousarray(xt),
                "cst": np.ascontiguousarray(cst),
            }
        )

    cfg = (fr, tp)
    if cfg not in _PROGRAM_CACHE:
        _PROGRAM_CACHE[cfg] = _build_program(cfg)
    nc = _PROGRAM_CACHE[cfg]

    res = run_bass_kernel_spmd(nc, in_maps, core_ids=list(range(N_CORES)))
    LAST_RESULTS = res
    LAST_IN_MAPS = in_maps
    LAST_NC = nc
    LAST_CFG = cfg

    # --- unshard: y_dev[p, s, k*512+n] -> point (4k+p)*1024 + s*512 + n ---
    y_sorted = np.empty(n_pts, dtype=np.float32)
    for e in range(E):
        ydev = res.results[e]["y"]             # [4, 2, nr*512]
        ye = np.empty((npairs, 1024), dtype=np.float32)
        for p in range(4):
            nq = fr + (1 if p < tp else 0)
            blk = ydev[p, :, : nq * F].reshape(2, nq, F)
            ye[p::4] = blk.transpose(1, 0, 2).reshape(nq, 1024)
        yflat = ye.reshape(cap)
        y_sorted[starts[e] : starts[e] + counts[e]] = yflat[: counts[e]]

    y_full = np.empty(n_pts, dtype=np.float32)
    y_full[order] = y_sorted
    return y_full[:, None]


# revision 26
# speedup vs baseline: 1.4849x; 1.0303x over previous
"""Ensemble-SRN MoE routing kernel for 8 TRN2 NeuronCores.

Strategy: expert-parallel sharding. The 8 experts are axis-aligned octants of
[-1,1]^3 (GRID=(2,2,2)); core e receives exactly the points routed to expert e
(the all-to-all dispatch happens on the host as part of sharding), runs a dense
single-expert 3->64->64->1 ReLU MLP over its (padded) shard, and the host
inverse-permutes the outputs.

Device kernel (per core, fr rounds of 4096 points = 4 pairs x 1024, plus a
tail of tp pairs sized from the actual max shard count):
  - All x coordinates are preloaded into SBUF once (4 DMAs, one per 6-row
    partition strip) and all y stays SBUF-resident until 2 waves of 4 output
    DMAs; per-round DMA traffic is zero.  (HWDGE costs ~625ns per dma_start,
    so per-round DMAs were the original top bottleneck.)
  - All weights/biases are packed into one 528-byte-per-partition uint8
    tensor, DMA'd once, and accessed through bitcast views.
  - L1 (3->64): 4 strip matmuls [6,128]x[6,512] at tile_position (32p, 0),
    two point-tiles block-diagonal per matmul.
  - L2 (64->64): 4 full-array matmuls with W2 block-diagonal twice.
  - L3 (64->1): 4 matmuls with M=2 block-diag w3 ([128,2]) at tile_position
    (0,32p); y for pair p lands at PSUM partitions {32p,32p+1} of bank 0 of
    ph2b (reused after its evacuation).
  - Evacuation split: DVE (tensor_scalar add-bias+relu) takes both L1
    halves; ACT (activation relu/identity with bias) takes both L2 halves
    plus the y evac.  Per round: DVE 2x[128,1024], ACT 2x[128,1024] +
    [128,512] - the ACT chain (~2.7us) is the steady-state critical path.
  - PSUM: ph1 pool 2x[128,1024]f32 + ph2 pool 2x[128,1024]f32 = all 8 banks;
    emission is software-pipelined (y evac of round i-1 ordered before round
    i's L2 evacs) so ACT streams without waiting on L3.
  - CAUTION: multi-partition-dim DMA APs (rearrange "(g sub) f -> g sub f")
    are silently wrong on hardware; only plain contiguous-partition slices
    are used for DMA.
"""

import ml_dtypes
import numpy as np

import concourse.bass as bass
import concourse.tile as tile
from concourse import bacc, mybir
from concourse.bass_utils import run_bass_kernel_spmd

F32 = mybir.dt.float32
BF16 = mybir.dt.bfloat16
U8 = mybir.dt.uint8

N_CORES = 8
GRID = (2, 2, 2)
H = 64
F = 512              # points per tile (one PSUM-bank free dim, fp32)

_PROGRAM_CACHE = {}
LAST_RESULTS = None  # BassKernelResults of the last run (for test harness)
LAST_IN_MAPS = None  # per-core input dicts of the last run (for test harness)
LAST_NC = None       # compiled program of the last run (for test harness)
LAST_CFG = None      # (fr, tp) of the last run (for test harness)


def _build_program(cfg, loop_n=None, stage="full"):
    """Build the SPMD program for cfg=(full_rounds, tail_pairs). loop_n
    (bench only): repeat the whole body loop_n times in a hardware For_i so
    device time can be measured through the noisy axon dispatch path by
    differencing two loop counts."""
    fr, tp = cfg
    nr = fr + (1 if tp else 0)          # column blocks in x/y SBUF tiles
    nc = bacc.Bacc(
        "TRN2",
        target_bir_lowering=False,
        debug=False,
        num_devices=N_CORES,
    )
    xT = nc.dram_tensor("xT", [4, 6, nr * F], BF16, kind="ExternalInput")
    # packed constants, one DMA: w1[0:256) w2[256:512) w3[512:516) b1[516:520)
    # b2[520:524) b3[524:528) bytes per partition
    cst = nc.dram_tensor("cst", [128, 528], U8, kind="ExternalInput")
    yO = nc.dram_tensor("y", [4, 2, nr * F], F32, kind="ExternalOutput")

    RELU = mybir.ActivationFunctionType.Relu
    IDENT = mybir.ActivationFunctionType.Identity
    ADD = mybir.AluOpType.add
    MAX = mybir.AluOpType.max

    with tile.TileContext(nc) as tc:
        with (
            tc.tile_pool(name="const", bufs=1) as const,
            tc.tile_pool(name="xall", bufs=1) as xall,
            tc.tile_pool(name="yall", bufs=1) as yall,
            tc.tile_pool(name="h1p", bufs=4) as h1pool,
            tc.tile_pool(name="h2p", bufs=4) as h2pool,
            tc.tile_pool(name="ps1", bufs=2, space="PSUM") as ps1,
            tc.tile_pool(name="ps2", bufs=2, space="PSUM") as ps2,
        ):
            c_sb = const.tile([128, 528], U8)
            nc.sync.dma_start(c_sb[:], cst.ap())
            w1_sb = c_sb[:, 0:256].bitcast(BF16)
            w2_sb = c_sb[:, 256:512].bitcast(BF16)
            w3_sb = c_sb[:, 512:516].bitcast(BF16)
            b1_sb = c_sb[:, 516:520].bitcast(F32)
            b2_sb = c_sb[:, 520:524].bitcast(F32)
            b3_sb = c_sb[:, 524:528].bitcast(F32)

            import contextlib
            loop_cm = (
                tc.For_i(
                    0, loop_n, 1,
                    hint_engines=(
                        mybir.EngineType.PE,
                        mybir.EngineType.DVE,
                        mybir.EngineType.Activation,
                        mybir.EngineType.SP,
                    ),
                )
                if loop_n
                else contextlib.nullcontext()
            )
            with loop_cm:
                x_sb = xall.tile([128, nr * F], BF16)
                # NOTE: multi-partition-dim APs (rearrange "(g sub) f") are
                # silently wrong through the DMA path on HW - use 4 plain
                # contiguous-partition DMAs.
                for p in range(4):
                    eng = nc.scalar if p % 2 == 0 else nc.sync
                    eng.dma_start(
                        x_sb[32 * p : 32 * p + 6, :], xT.ap()[p]
                    )
                y_sb = yall.tile([128, nr * F], F32)

                # constant h tiles for stage="pe" (decouple MMs from evacs)
                if stage == "pe":
                    h1c = const.tile([128, 2048], BF16)
                    nc.vector.memset(h1c[:], 1.0)
                    h2c = const.tile([128, 2048], BF16)
                    nc.vector.memset(h2c[:], 1.0)

                rounds = [(r, 4) for r in range(fr)]
                if tp:
                    rounds.append((fr, tp))
                n = len(rounds)
                S = [dict() for _ in range(n)]

                def emit_L1(i):
                    r, pairs = rounds[i]
                    c0 = r * F
                    if pairs == 4:
                        ph1a = ps1.tile([128, 1024], F32, tag="hp1")
                        ph1b = ps1.tile([128, 1024], F32, tag="hp1")
                        ph1 = [ph1a, ph1b]
                    else:
                        ph1t = ps1.tile([128, F * pairs], F32, tag="hp1")
                        ph1 = [ph1t]
                    S[i].update(ph1=ph1, c0=c0, pairs=pairs)
                    if stage != "evac":
                        for p in range(pairs):
                            nc.tensor.matmul(
                                ph1[p // 2][:, F * (p % 2) : F * (p % 2) + F],
                                w1_sb[32 * p : 32 * p + 6, :],
                                x_sb[32 * p : 32 * p + 6, c0 : c0 + F],
                                start=True,
                                stop=True,
                                tile_position=(32 * p, 0),
                            )

                def emit_evac1(i):  # DVE: bias+relu for both L1 halves
                    pairs = S[i]["pairs"]
                    ph1 = S[i]["ph1"]
                    h1r = h1pool.tile([128, F * pairs], BF16, tag="h1r")
                    S[i]["h1r"] = h1r
                    if stage == "pe":
                        return
                    if pairs == 4:
                        nc.vector.tensor_scalar(
                            h1r[:, 0:1024], ph1[0][:], b1_sb[:, 0:1],
                            0.0, ADD, MAX,
                        )
                        nc.vector.tensor_scalar(
                            h1r[:, 1024:2048], ph1[1][:], b1_sb[:, 0:1],
                            0.0, ADD, MAX,
                        )
                    else:
                        nc.vector.tensor_scalar(
                            h1r[:], ph1[0][:], b1_sb[:, 0:1], 0.0, ADD, MAX,
                        )

                def emit_L2(i):
                    pairs = S[i]["pairs"]
                    h1in = h1c if stage == "pe" else S[i]["h1r"]
                    if pairs == 4:
                        ph2a = ps2.tile([128, 1024], F32, tag="hp2")
                        ph2b = ps2.tile([128, 1024], F32, tag="hp2")
                        ph2 = [ph2a, ph2b]
                    else:
                        ph2t = ps2.tile([128, F * pairs], F32, tag="hp2")
                        ph2 = [ph2t]
                    S[i]["ph2"] = ph2
                    if stage != "evac":
                        for p in range(pairs):
                            nc.tensor.matmul(
                                ph2[p // 2][:, F * (p % 2) : F * (p % 2) + F],
                                w2_sb[:, :],
                                h1in[:, F * p : F * p + F],
                                start=True,
                                stop=True,
                                tile_position=(0, 0),
                            )

                def emit_evac2(i):  # ACT: bias+relu for both L2 halves
                    pairs = S[i]["pairs"]
                    ph2 = S[i]["ph2"]
                    h2r = h2pool.tile([128, F * pairs], BF16, tag="h2r")
                    S[i]["h2r"] = h2r
                    if stage == "pe":
                        return
                    if pairs == 4:
                        nc.scalar.activation(
                            h2r[:, 0:1024], ph2[0][:], RELU,
                            bias=b2_sb[:, 0:1],
                        )
                        nc.scalar.activation(
                            h2r[:, 1024:2048], ph2[1][:], RELU,
                            bias=b2_sb[:, 0:1],
                        )
                    else:
                        nc.vector.tensor_scalar(
                            h2r[:], ph2[0][:], b2_sb[:, 0:1], 0.0, ADD, MAX,
                        )

                def emit_L3(i):
                    pairs = S[i]["pairs"]
                    h2in = h2c if stage == "pe" else S[i]["h2r"]
                    yt = S[i]["ph2"][-1]
                    if stage != "evac":
                        for p in range(pairs):
                            nc.tensor.matmul(
                                yt[32 * p : 32 * p + 2, 0:F],
                                w3_sb[:, 0:2],
                                h2in[:, F * p : F * p + F],
                                start=True,
                                stop=True,
                                tile_position=(0, 32 * p),
                            )

                def emit_y(i):  # ACT: bias-add copy of y psum into y_sb
                    if stage == "pe":
                        return
                    yt = S[i]["ph2"][-1]
                    c0 = S[i]["c0"]
                    nc.scalar.activation(
                        y_sb[:, c0 : c0 + F], yt[:, 0:F],
                        IDENT, bias=b3_sb[:, 0:1]
                    )

                h = (3 * fr) // 4  # first y-DMA wave covers rounds 0..h-1
                if stage == "pe":
                    h = 0  # y_sb never written; tiny debug output instead
                # software-pipelined emission: each engine's queue is ordered
                # so the head is (expected) always ready:
                #   PE : L1(0), L2(0), L1(1), L3(0), L2(1), L1(2), L3(1), ...
                #   DVE: L1a(i), L1b(i)
                #   ACT: y(i-1), L2a(i), L2b(i)
                emit_L1(0)
                for i in range(n):
                    emit_evac1(i)
                    if i >= 1:
                        emit_y(i - 1)
                    if i == h and h > 0:
                        for g in range(4):
                            nc.sync.dma_start(
                                yO.ap()[g][:, 0 : h * F],
                                y_sb[32 * g : 32 * g + 2, 0 : h * F],
                            )
                    emit_L2(i)
                    if i + 1 < n:
                        emit_L1(i + 1)
                    emit_evac2(i)
                    emit_L3(i)
                emit_y(n - 1)

                if stage == "pe":
                    nc.sync.dma_start(
                        yO.ap()[0][:, 0:16], h1c[0:2, 0:32].bitcast(F32)
                    )
                else:
                    for g in range(4):
                        eng = nc.sync if g % 2 == 0 else nc.scalar
                        eng.dma_start(
                            yO.ap()[g][:, h * F :],
                            y_sb[32 * g : 32 * g + 2, h * F :],
                        )

    nc.compile()
    return nc


def kernel(x, extents_min, extents_max, W1, b1, W2, b2, W3, b3):
    global LAST_RESULTS, LAST_IN_MAPS, LAST_NC, LAST_CFG
    x = np.ascontiguousarray(np.asarray(x, dtype=np.float32))
    extents_min = np.asarray(extents_min, dtype=np.float32)
    extents_max = np.asarray(extents_max, dtype=np.float32)
    W1 = np.asarray(W1, dtype=np.float32)
    b1 = np.asarray(b1, dtype=np.float32)
    W2 = np.asarray(W2, dtype=np.float32)
    b2 = np.asarray(b2, dtype=np.float32)
    W3 = np.asarray(W3, dtype=np.float32)
    b3 = np.asarray(b3, dtype=np.float32)

    n_pts = x.shape[0]
    E = W1.shape[0]
    assert E == N_CORES

    # --- routing (identical fp32 math to the reference) ---
    gvec = np.asarray(GRID, dtype=np.float32)
    u = np.clip((x + np.float32(1.0)) * np.float32(0.5), 0.0, 0.99)
    gi = (u * gvec).astype(np.int32)
    idx = gi[:, 0] + gi[:, 1] * GRID[0] + gi[:, 2] * (GRID[0] * GRID[1])

    order = np.argsort(idx, kind="stable")
    counts = np.bincount(idx, minlength=E)
    starts = np.concatenate([[0], np.cumsum(counts)[:-1]])
    x_sorted = x[order]

    maxc = int(counts.max())
    fr = maxc // 4096
    tp = -(-(maxc - fr * 4096) // 1024)  # ceil
    if tp == 4:
        fr, tp = fr + 1, 0
    if fr == 0 and tp == 0:
        tp = 1
    cap = fr * 4096 + tp * 1024
    npairs = fr * 4 + tp
    nr = fr + (1 if tp else 0)

    # --- fold the expert-local normalization into layer-1 weights ---
    # xn = s*x + t, s = 2/(emax-emin), t = -2*emin/(emax-emin) - 1
    span = extents_max - extents_min          # [E, 3]
    s = 2.0 / span
    tvec = -2.0 * extents_min / span - 1.0
    W1p = W1 * s[:, :, None]                  # [E, 3, H]
    b1p = b1 + np.einsum("ec,ech->eh", tvec, W1)

    in_maps = []
    for e in range(E):
        xe = np.zeros((cap, 3), dtype=np.float32)
        xe[: counts[e]] = x_sorted[starts[e] : starts[e] + counts[e]]
        # xT[p, 3s+c, k*512+n] = xe[(4k+p)*1024 + s*512 + n, c]
        xq = xe.reshape(npairs, 2, F, 3)
        xt = np.zeros((4, 6, nr * F), dtype=ml_dtypes.bfloat16)
        for p in range(4):
            nq = fr + (1 if p < tp else 0)
            blk = xq[p::4]                     # [nq, 2, 512, 3]
            assert blk.shape[0] == nq
            # rows 3s+c, cols k*512+n
            xt[p, :, : nq * F] = (
                blk.transpose(1, 3, 0, 2).reshape(6, nq * F)
                .astype(ml_dtypes.bfloat16)
            )
        # w1: 4 row strips (one per pair), each the [6,128] block-diag of W1'
        w1e = W1p[e].astype(ml_dtypes.bfloat16)
        w1_full = np.zeros((128, 128), dtype=ml_dtypes.bfloat16)
        for p in range(4):
            w1_full[32 * p : 32 * p + 3, 0:64] = w1e
            w1_full[32 * p + 3 : 32 * p + 6, 64:128] = w1e
        # w2: [128,128] block-diag of W2
        w2_full = np.zeros((128, 128), dtype=ml_dtypes.bfloat16)
        w2_full[0:64, 0:64] = W2[e].astype(ml_dtypes.bfloat16)
        w2_full[64:128, 64:128] = W2[e].astype(ml_dtypes.bfloat16)
        # w3: [128,2] block-diag
        w3_full = np.zeros((128, 2), dtype=ml_dtypes.bfloat16)
        w3_full[0:64, 0] = W3[e, :, 0].astype(ml_dtypes.bfloat16)
        w3_full[64:128, 1] = W3[e, :, 0].astype(ml_dtypes.bfloat16)
        b1_full = np.tile(b1p[e], 2)[:, None].astype(np.float32)
        b2_full = np.tile(b2[e], 2)[:, None].astype(np.float32)
        b3_full = np.full((128, 1), b3[e, 0], dtype=np.float32)
        cst = np.concatenate(
            [
                w1_full.view(np.uint8),
                w2_full.view(np.uint8),
                w3_full.view(np.uint8),
                b1_full.view(np.uint8),
                b2_full.view(np.uint8),
                b3_full.view(np.uint8),
            ],
            axis=1,
        )
        assert cst.shape == (128, 528), cst.shape
        in_maps.append(
            {
                "xT": np.ascontiguousarray(xt),
                "cst": np.ascontiguousarray(cst),
            }
        )

    cfg = (fr, tp)
    if cfg not in _PROGRAM_CACHE:
        _PROGRAM_CACHE[cfg] = _build_program(cfg)
    nc = _PROGRAM_CACHE[cfg]

    res = run_bass_kernel_spmd(nc, in_maps, core_ids=list(range(N_CORES)))
    LAST_RESULTS = res
    LAST_IN_MAPS = in_maps
    LAST_NC = nc
    LAST_CFG = cfg

    # --- unshard: y_dev[p, s, k*512+n] -> point (4k+p)*1024 + s*512 + n ---
    y_sorted = np.empty(n_pts, dtype=np.float32)
    for e in range(E):
        ydev = res.results[e]["y"]             # [4, 2, nr*512]
        ye = np.empty((npairs, 1024), dtype=np.float32)
        for p in range(4):
            nq = fr + (1 if p < tp else 0)
            blk = ydev[p, :, : nq * F].reshape(2, nq, F)
            ye[p::4] = blk.transpose(1, 0, 2).reshape(nq, 1024)
        yflat = ye.reshape(cap)
        y_sorted[starts[e] : starts[e] + counts[e]] = yflat[: counts[e]]

    y_full = np.empty(n_pts, dtype=np.float32)
    y_full[order] = y_sorted
    return y_full[:, None]
